# revision 4
# baseline (speedup 1.0000x reference)
"""Causal self-attention kernel for Trainium2 (8 NeuronCores, data-parallel).

Problem: B=8, T=2048, C=1024 single-head causal attention:
    qkv = x @ w_attn + b_attn ; q,k,v = split(qkv)
    attn = softmax(q @ k.T / sqrt(C) + causal_mask)
    out  = (attn @ v) @ w_proj + b_proj

Sharding: pure data parallel — one batch element per core, weights replicated,
no collectives.

Per-core algorithm (all matmuls bf16 operands, fp32 PSUM accumulate):
  host: xT = x[b].T cast bf16 (so the contraction dim is the partition dim
        everywhere on device; no on-device transposes needed anywhere).
  ph1:  qT[e,t], kT[e,s]  <- matmul(lhsT=w_qk[c,e-tile], rhs=xT[c,t])   [e,t] layout
        v[t,c']           <- matmul(lhsT=xT[c,t-tile],  rhs=w_v[c,c'])  natural layout
  ph2:  per 512-wide t-chunk ("supertile"), per 128-wide s-tile (causal only):
        ST[s,t]  <- matmul(lhsT=kT[e,s-tile], rhs=qT[e,t-chunk])  (8 e-tiles acc)
        P~T[s,t] <- exp(ST/sqrt(C) + mask)   (no max-subtract; logits are O(1))
        sums[t]  <- matmul(lhsT=ones[s,1], rhs=P~T)  (acc over s-tiles)
        OT[c',t] <- matmul(lhsT=v[s-tile,c'-tile], rhs=P~T[s-tile,t-chunk])
        out[t,d] <- matmul(lhsT=OT[c',t-tile], rhs=w_proj[c',d]) * (1/sums[t])
  The 1/sums normalization is folded into the final PSUM->SBUF copy as a
  per-partition activation scale (everything between exp and out is linear).
  The proj stage runs one supertile behind (software pipeline) so the
  sums->reciprocal DMA roundtrip never stalls the PE.

DMA plan (3 queues: sync + scalar are HWDGE, gpsimd is SWDGE):
  - xT c-tiles split by parity across sync/scalar, each tile DMA'd as two
    1024-col halves so the first matmuls can start ~4us in.
  - weights live in single 3-dim SBUF tiles ([P, c, cols]) so each eb-block
    is ONE big descriptor (per-descriptor issue is ~0.65us on a queue;
    32 small descriptors would serialize for 20us+).
  - wqk eb0 goes per-c-tile first (matches the c-outer e-group 0-1 ramp
    ordering), the rest as 1MB descriptors.
  - No PE warmup: the Tensor queue's framework preamble ends ~6.5us anyway,
    by which time the first xT/wqk blocks have landed; the p-state clock
    ramps during the DMA-limited opening instead of on dummy matmuls.

Output is written bf16 (halves the out DMA) and upcast on host.

b_attn is folded in by augmenting x with a ones column (padded to a full
128-partition tile) only when it is nonzero; b_proj is added on the host.
"""

import sys

if "/opt/trn_rl_repo" not in sys.path:
    sys.path.insert(0, "/opt/trn_rl_repo")

import numpy as np
import ml_dtypes

import concourse.bacc as bacc
import concourse.mybir as mybir
import concourse.tile as tile
from concourse.bass_utils import run_bass_kernel_spmd

B, T, C = 8, 2048, 1024
P = 128  # partitions
TCH = 512  # t-chunk (moving free dim)
N_TT = T // P  # 16 t-tiles
N_SUP = T // TCH  # 4 supertiles
N_ET = C // P  # 8 e-tiles (q/k feature dim)
SCALE = 1.0 / float(np.sqrt(np.float32(C)))
NEG = -10000000000.0

BF16 = mybir.dt.bfloat16
FP32 = mybir.dt.float32

_cache = {}


def _build(n_ct):
    """Build the SPMD Bass program. n_ct = number of 128-wide c-tiles of the
    (possibly ones-augmented) input feature dim."""
    nc = bacc.Bacc("TRN2", target_bir_lowering=False, debug=False, num_devices=8)

    xT_d = nc.dram_tensor("xT", [n_ct * P, T], BF16, kind="ExternalInput").ap()
    wqk_d = nc.dram_tensor("wqk", [n_ct, P, 2 * C], BF16, kind="ExternalInput").ap()
    wv_d = nc.dram_tensor("wv", [n_ct, P, C], BF16, kind="ExternalInput").ap()
    wp_d = nc.dram_tensor("wp", [N_ET, P, C], BF16, kind="ExternalInput").ap()
    maskT_d = nc.dram_tensor("maskT", [P, P], FP32, kind="ExternalInput").ap()
    out_d = nc.dram_tensor("out", [T, C], BF16, kind="ExternalOutput").ap()
    scr_d = nc.dram_tensor("scr", [N_SUP, 4, TCH], FP32, kind="ExternalOutput").ap()

    with tile.TileContext(nc) as tc:
        with (
            tc.tile_pool(name="persist", bufs=1) as persist,
            tc.tile_pool(name="small", bufs=1) as small,
        ):
            ones = small.tile([P, 1], BF16, name="ones", tag="ones")
            warm_in = small.tile([P, TCH], BF16, name="warm_in", tag="warm_in")
            nc.vector.memset(ones[:], 1.0)
            nc.vector.memset(warm_in[:], 0.0)
            # PE warmup: ~22 wide (512-col) matmuls keep the PE busy from the
            # end of the framework preamble (~7us) until the first input
            # descriptors complete (~12us), ramping the HAM clock to 2.4 GHz.
            # Wide matmuls so the stretch is matmul-bound, not issue-bound.
            with tc.tile_pool(name="warm_ps", bufs=1, space="PSUM") as warm_ps:
                wps = warm_ps.tile([1, TCH], FP32, name="wps", tag="wps")
                for _ in range(22):
                    nc.tensor.matmul(wps[:], ones[:], warm_in[:], start=True, stop=True)

            # persistent SBUF arrays
            qT = [persist.tile([P, T], BF16, name=f"qT{e}", tag=f"qT{e}") for e in range(N_ET)]
            kT = [persist.tile([P, T], BF16, name=f"kT{e}", tag=f"kT{e}") for e in range(N_ET)]
            v = [persist.tile([P, C], BF16, name=f"v{t}", tag=f"v{t}") for t in range(N_TT)]
            maskT = small.tile([P, P], FP32, name="maskT", tag="maskT")
            # w_proj persists into phase 2; loaded early on the scalar queue.
            wp = persist.tile([P, N_ET, C], BF16, name="wp", tag="wp")

            # ---------------- phase 1: projections ----------------
            with (
                tc.tile_pool(name="ph1", bufs=1) as ph1,
                tc.tile_pool(name="ph1ps", bufs=8, space="PSUM") as ph1ps,
            ):
                xT = [ph1.tile([P, T], BF16, name=f"xT{c}", tag=f"xT{c}") for c in range(n_ct)]
                wqk = ph1.tile([P, n_ct, 2 * C], BF16, name="wqk", tag="wqk")
                wv = ph1.tile([P, n_ct, C], BF16, name="wv", tag="wv")

                # gpsimd (SWDGE): wqk. eb0 cols 0:256 per-c (feeds e-groups
                # 0-1 c-outer ASAP), then big 1-descriptor blocks.
                for c in range(n_ct):
                    nc.gpsimd.dma_start(wqk[:, c, :256], wqk_d[c, :, :256])
                nc.gpsimd.dma_start(
                    wqk[:, :, 256:512],
                    wqk_d[:, :, 256:512].rearrange("c p e -> p c e"),
                )
                for eb in range(1, 2 * C // TCH):
                    nc.gpsimd.dma_start(
                        wqk[:, :, eb * TCH : (eb + 1) * TCH],
                        wqk_d[:, :, eb * TCH : (eb + 1) * TCH].rearrange(
                            "c p e -> p c e"
                        ),
                    )

                # xT: parity split across sync/scalar HWDGE queues, two
                # 1024-col halves per c-tile (finer arrival granularity).
                ch = [c for c in range(n_ct) if c % 2 == 0]
                co = [c for c in range(n_ct) if c % 2 == 1]
                for h in range(2):
                    hs = slice(h * (T // 2), (h + 1) * (T // 2))
                    for i in range(max(len(ch), len(co))):
                        if i < len(ch):
                            c = ch[i]
                            nc.sync.dma_start(
                                xT[c][:, hs], xT_d[c * P : (c + 1) * P, hs]
                            )
                        if i < len(co):
                            c = co[i]
                            nc.scalar.dma_start(
                                xT[c][:, hs], xT_d[c * P : (c + 1) * P, hs]
                            )
                # interleave halves per c so both halves of c arrive before
                # c+2's first half is needed: reorder -> (c,h0),(c,h1) per c
                # (handled by the loop above emitting h0 for all c then h1;
                # supply comfortably outruns the 1.7us/c-tile consumption)

                # wv: halves on sync/scalar after xT; wp on scalar; mask last.
                half = (n_ct + 1) // 2
                nc.sync.dma_start(
                    wv[:, :half, :], wv_d[:half].rearrange("c p e -> p c e")
                )
                nc.scalar.dma_start(
                    wv[:, half:, :], wv_d[half:].rearrange("c p e -> p c e")
                )
                nc.scalar.dma_start(wp[:], wp_d.rearrange("c p e -> p c e"))
                nc.sync.dma_start(maskT[:], maskT_d[:])

                # qT/kT: psum[e-tile, t-chunk] = sum_c w_qk[c, e].T @ xT[c, t]
                # Opening: e-groups 0-3 in two c-OUTER passes over the xT
                # halves (pass A: tc 0-1 needs only each tile's first 1024
                # cols, pass B: tc 2-3).  4 e-groups x 2 chunks = 8 PSUM
                # banks per pass, and each arriving 256KB half feeds 4096
                # PE cycles (~150 GB/s demand, matching 2-queue delivery).
                for h in range(2):
                    pss = [
                        [
                            ph1ps.tile([P, TCH], FP32, name="qkps01", tag="qkps")
                            for _ in range(2)
                        ]
                        for _ in range(4)
                    ]
                    for c in range(n_ct):
                        for e in range(4):
                            for ti in range(2):
                                tc_i = 2 * h + ti
                                nc.tensor.matmul(
                                    pss[e][ti][:],
                                    wqk[:, c, e * P : (e + 1) * P],
                                    xT[c][:, tc_i * TCH : (tc_i + 1) * TCH],
                                    start=(c == 0),
                                    stop=(c == n_ct - 1),
                                )
                    for e in range(4):
                        for ti in range(2):
                            tc_i = 2 * h + ti
                            dst_ap = qT[e][:, tc_i * TCH : (tc_i + 1) * TCH]
                            if (e * 4 + tc_i) % 2 == 0:
                                nc.vector.tensor_copy(dst_ap, pss[e][ti][:])
                            else:
                                nc.scalar.copy(dst_ap, pss[e][ti][:])

                for e in range(4, 2 * N_ET):
                    dst = qT[e] if e < N_ET else kT[e - N_ET]
                    pss = [
                        ph1ps.tile([P, TCH], FP32, name="qkps", tag="qkps")
                        for _ in range(T // TCH)
                    ]
                    for c in range(n_ct):
                        for tc_i in range(T // TCH):
                            nc.tensor.matmul(
                                pss[tc_i][:],
                                wqk[:, c, e * P : (e + 1) * P],
                                xT[c][:, tc_i * TCH : (tc_i + 1) * TCH],
                                start=(c == 0),
                                stop=(c == n_ct - 1),
                            )
                    for tc_i in range(T // TCH):
                        dst_ap = dst[:, tc_i * TCH : (tc_i + 1) * TCH]
                        if (e * 4 + tc_i) % 2 == 0:
                            nc.vector.tensor_copy(dst_ap, pss[tc_i][:])
                        else:
                            nc.scalar.copy(dst_ap, pss[tc_i][:])

                # v: psum[t-tile, c'-chunk] = sum_c xT[c, t].T @ w_v[c, c']
                for t in range(N_TT):
                    pss = [
                        ph1ps.tile([P, TCH], FP32, name="vps", tag="qkps")
                        for _ in range(C // TCH)
                    ]
                    for c in range(n_ct):
                        for cc in range(C // TCH):
                            nc.tensor.matmul(
                                pss[cc][:],
                                xT[c][:, t * P : (t + 1) * P],
                                wv[:, c, cc * TCH : (cc + 1) * TCH],
                                start=(c == 0),
                                stop=(c == n_ct - 1),
                            )
                    for cc in range(C // TCH):
                        dst_ap = v[t][:, cc * TCH : (cc + 1) * TCH]
                        if (t * 2 + cc) % 2 == 0:
                            nc.vector.tensor_copy(dst_ap, pss[cc][:])
                        else:
                            nc.scalar.copy(dst_ap, pss[cc][:])

            # ---------------- phase 2: attention + proj ----------------
            with (
                tc.tile_pool(name="pt_pool", bufs=18) as pt_pool,
                tc.tile_pool(name="ot_pool", bufs=3) as ot_pool,
                tc.tile_pool(name="stage", bufs=3) as stage,
                tc.tile_pool(name="st_ps", bufs=2, space="PSUM") as st_ps,
                tc.tile_pool(name="sums_ps", bufs=1, space="PSUM") as sums_ps,
                tc.tile_pool(name="ot_ps", bufs=2, space="PSUM") as ot_ps,
                tc.tile_pool(name="pr_ps", bufs=3, space="PSUM") as pr_ps,
            ):
                def emit_proj(t0, ot_sb, rt):
                    """proj for the supertile starting at t0, scaled by 1/sums.
                    dch-outer so each 512-wide output chunk's copy+DMA starts
                    while the next chunk's matmuls still run (lean tail)."""
                    for k in range(TCH // P):  # t-tile within supertile
                        for dch in range(C // TCH):
                            prs = pr_ps.tile([P, TCH], FP32, name="pr", tag="pr")
                            for g in range(N_ET):
                                nc.tensor.matmul(
                                    prs[:],
                                    ot_sb[g][:, k * P : (k + 1) * P],
                                    wp[:, g, dch * TCH : (dch + 1) * TCH],
                                    start=(g == 0),
                                    stop=(g == N_ET - 1),
                                )
                            osb_out = stage.tile([P, TCH], BF16, name="osb_out", tag="osb_out")
                            if dch % 2 == 0:
                                nc.scalar.activation(
                                    osb_out[:],
                                    prs[:],
                                    mybir.ActivationFunctionType.Copy,
                                    scale=rt[:, k : k + 1],
                                )
                            else:
                                nc.vector.tensor_scalar_mul(
                                    osb_out[:], prs[:], rt[:, k : k + 1]
                                )
                            nc.sync.dma_start(
                                out_d[
                                    t0 + k * P : t0 + (k + 1) * P,
                                    dch * TCH : (dch + 1) * TCH,
                                ],
                                osb_out[:],
                            )

                pending = None  # (t0, ot_sb, rt) of the previous supertile
                for i in range(N_SUP):  # supertile: t in [i*TCH, (i+1)*TCH)
                    t0 = i * TCH
                    n_st = 4 * i + 4  # causal s-tiles
                    ptiles = []
                    # --- ST + exp per s-tile ---
                    for j in range(n_st):
                        off = max(0, j - 4 * i) * P  # first valid t column
                        st = st_ps.tile([P, TCH], FP32, name="st", tag="st")
                        for e in range(N_ET):
                            nc.tensor.matmul(
                                st[:, off:TCH],
                                kT[e][:, j * P : (j + 1) * P],
                                qT[e][:, t0 + off : t0 + TCH],
                                start=(e == 0),
                                stop=(e == N_ET - 1),
                            )
                        if j >= 4 * i:  # diagonal block: strict-upper (s>t) mask
                            nc.vector.tensor_add(
                                st[:, off : off + P], st[:, off : off + P], maskT[:]
                            )
                        pt = pt_pool.tile([P, TCH], BF16, name="pt", tag="pt")
                        nc.scalar.activation(
                            pt[:, off:TCH],
                            st[:, off:TCH],
                            mybir.ActivationFunctionType.Exp,
                            scale=SCALE,
                        )
                        ptiles.append((pt, off))

                    # --- row sums via ones-matmul (acc over s-tiles) ---
                    # j=0 always has off=0, so the first (start=True) matmul
                    # covers the full width; later partial-width matmuls
                    # accumulate into their column subrange only.
                    sums = sums_ps.tile([1, TCH], FP32, name="sums", tag="sums")
                    for j in range(n_st):
                        pt, off = ptiles[j]
                        nc.tensor.matmul(
                            sums[:, off:TCH],
                            ones[:],
                            pt[:, off:TCH],
                            start=(j == 0),
                            stop=(j == n_st - 1),
                        )
                    srow = stage.tile([1, TCH], FP32, name="srow", tag="srow")
                    nc.vector.tensor_copy(srow[:], sums[:])
                    nc.sync.dma_start(scr_d[i, 0:1, :], srow[:])
                    rt0 = stage.tile([P, N_SUP], FP32, name="rt0", tag="rt0")
                    nc.sync.dma_start(
                        rt0[:], scr_d[i, 0].rearrange("(f q) -> q f", q=P)
                    )
                    rt = stage.tile([P, N_SUP], FP32, name="rt", tag="rt")
                    nc.vector.reciprocal(rt[:], rt0[:])

                    # --- previous supertile's proj (hides the recip roundtrip) ---
                    if pending is not None:
                        emit_proj(*pending)

                    # --- OT[c'-tile, t-chunk] = sum_s v[s,c'].T @ P~T[s,t] ---
                    ot_sb = []
                    for g in range(N_ET):
                        ot = ot_ps.tile([P, TCH], FP32, name="ot", tag="ot")
                        for j in range(n_st):
                            pt, off = ptiles[j]
                            nc.tensor.matmul(
                                ot[:, off:TCH],
                                v[j][:, g * P : (g + 1) * P],
                                pt[:, off:TCH],
                                start=(j == 0),
                                stop=(j == n_st - 1),
                            )
                        osb = ot_pool.tile([P, TCH], BF16, name="osb", tag=f"osb{g % 3}")
                        nc.vector.tensor_copy(osb[:], ot[:])
                        ot_sb.append(osb)

                    pending = (t0, ot_sb, rt)

                emit_proj(*pending)

    nc.compile()
    return nc


def kernel(x, w_attn, b_attn, w_proj, b_proj):
    x = np.asarray(x, dtype=np.float32)
    w_attn = np.asarray(w_attn, dtype=np.float32)
    b_attn = np.asarray(b_attn, dtype=np.float32)
    w_proj = np.asarray(w_proj, dtype=np.float32)
    b_proj = np.asarray(b_proj, dtype=np.float32)
    assert x.shape == (B, T, C)

    aug = bool(np.any(b_attn != 0.0))
    n_ct = C // P + (1 if aug else 0)
    if n_ct not in _cache:
        _cache[n_ct] = _build(n_ct)
    nc = _cache[n_ct]

    bf = ml_dtypes.bfloat16
    if aug:
        wqk = np.zeros((n_ct, P, 2 * C), dtype=bf)
        wqk.reshape(n_ct * P, 2 * C)[:C] = w_attn[:, : 2 * C].astype(bf)
        wqk.reshape(n_ct * P, 2 * C)[C] = b_attn[: 2 * C].astype(bf)
        wv = np.zeros((n_ct, P, C), dtype=bf)
        wv.reshape(n_ct * P, C)[:C] = w_attn[:, 2 * C :].astype(bf)
        wv.reshape(n_ct * P, C)[C] = b_attn[2 * C :].astype(bf)
    else:
        wqk = np.ascontiguousarray(w_attn[:, : 2 * C]).astype(bf).reshape(n_ct, P, 2 * C)
        wv = np.ascontiguousarray(w_attn[:, 2 * C :]).astype(bf).reshape(n_ct, P, C)
    wp = w_proj.astype(bf).reshape(N_ET, P, C)

    # strict upper triangle (s > t) additive mask for transposed [s, t] blocks
    maskT = np.where(
        np.arange(P)[:, None] > np.arange(P)[None, :], np.float32(NEG), np.float32(0.0)
    ).astype(np.float32)

    in_maps = []
    for b in range(B):
        xT = np.ascontiguousarray(x[b].T).astype(bf)
        if aug:
            xTa = np.zeros((n_ct * P, T), dtype=bf)
            xTa[:C] = xT
            xTa[C] = bf(1.0)
            xT = xTa
        in_maps.append({"xT": xT, "wqk": wqk, "wv": wv, "wp": wp, "maskT": maskT})

    global _last_in_maps
    _last_in_maps = in_maps
    res = run_bass_kernel_spmd(nc, in_maps, core_ids=list(range(8)))
    out = np.stack([res.results[b]["out"] for b in range(B)]).astype(np.float32)
    if np.any(b_proj != 0.0):
        out = out + b_proj[None, None, :]
    return out


if __name__ == "__main__":
    rng = np.random.default_rng(0)
    x = rng.standard_normal((B, T, C), dtype=np.float32)
    w_attn = rng.standard_normal((C, 3 * C), dtype=np.float32) / np.sqrt(C)
    b_attn = np.zeros(3 * C, dtype=np.float32)
    w_proj = rng.standard_normal((C, C), dtype=np.float32) / np.sqrt(C)
    b_proj = np.zeros(C, dtype=np.float32)
    out = kernel(x, w_attn, b_attn, w_proj, b_proj)
    print(out.shape, out.dtype)


# revision 6
# speedup vs baseline: 1.0289x; 1.0289x over previous
"""Causal self-attention kernel for Trainium2 (8 NeuronCores, data-parallel).

Problem: B=8, T=2048, C=1024 single-head causal attention:
    qkv = x @ w_attn + b_attn ; q,k,v = split(qkv)
    attn = softmax(q @ k.T / sqrt(C) + causal_mask)
    out  = (attn @ v) @ w_proj + b_proj

Sharding: pure data parallel — one batch element per core, weights replicated,
no collectives.

Per-core algorithm (all matmuls bf16 operands, fp32 PSUM accumulate):
  host: xT = x[b].T cast bf16 (so the contraction dim is the partition dim
        everywhere on device; no on-device transposes needed anywhere).
  ph1:  qT[e,t], kT[e,s]  <- matmul(lhsT=w_qk[c,e-tile], rhs=xT[c,t])   [e,t] layout
        v[t,c']           <- matmul(lhsT=xT[c,t-tile],  rhs=w_v[c,c'])  natural layout
  ph2:  per 512-wide t-chunk ("supertile"), per 128-wide s-tile (causal only):
        ST[s,t]  <- matmul(lhsT=kT[e,s-tile], rhs=qT[e,t-chunk])  (8 e-tiles acc)
        P~T[s,t] <- exp(ST/sqrt(C) + mask)   (no max-subtract; logits are O(1))
        sums[t]  <- matmul(lhsT=ones[s,1], rhs=P~T)  (acc over s-tiles)
        OT[c',t] <- matmul(lhsT=v[s-tile,c'-tile], rhs=P~T[s-tile,t-chunk])
        out[t,d] <- matmul(lhsT=OT[c',t-tile], rhs=w_proj[c',d]) * (1/sums[t])
  The 1/sums normalization is folded into the final PSUM->SBUF copy as a
  per-partition activation scale (everything between exp and out is linear).
  The proj stage runs one supertile behind (software pipeline) so the
  sums->reciprocal DMA roundtrip never stalls the PE.

DMA plan (3 queues: sync + scalar are HWDGE, gpsimd is SWDGE):
  - xT c-tiles split by parity across sync/scalar, each tile DMA'd as two
    1024-col halves so the first matmuls can start ~4us in.
  - weights live in single 3-dim SBUF tiles ([P, c, cols]) so each eb-block
    is ONE big descriptor (per-descriptor issue is ~0.65us on a queue;
    32 small descriptors would serialize for 20us+).
  - wqk eb0 goes per-c-tile first (matches the c-outer e-group 0-1 ramp
    ordering), the rest as 1MB descriptors.
  - No PE warmup: the Tensor queue's framework preamble ends ~6.5us anyway,
    by which time the first xT/wqk blocks have landed; the p-state clock
    ramps during the DMA-limited opening instead of on dummy matmuls.

Output is written bf16 (halves the out DMA) and upcast on host.

b_attn is folded in by augmenting x with a ones column (padded to a full
128-partition tile) only when it is nonzero; b_proj is added on the host.
"""

import sys

if "/opt/trn_rl_repo" not in sys.path:
    sys.path.insert(0, "/opt/trn_rl_repo")

import numpy as np
import ml_dtypes

import concourse.bacc as bacc
import concourse.mybir as mybir
import concourse.tile as tile
from concourse.bass_utils import run_bass_kernel_spmd

B, T, C = 8, 2048, 1024
P = 128  # partitions
TCH = 512  # t-chunk (moving free dim)
N_TT = T // P  # 16 t-tiles
N_SUP = T // TCH  # 4 supertiles
N_ET = C // P  # 8 e-tiles (q/k feature dim)
SCALE = 1.0 / float(np.sqrt(np.float32(C)))
NEG = -10000000000.0

BF16 = mybir.dt.bfloat16
FP32 = mybir.dt.float32

_cache = {}


def _build(n_ct):
    """Build the SPMD Bass program. n_ct = number of 128-wide c-tiles of the
    (possibly ones-augmented) input feature dim."""
    nc = bacc.Bacc("TRN2", target_bir_lowering=False, debug=False, num_devices=8)

    xT_d = nc.dram_tensor("xT", [n_ct * P, T], BF16, kind="ExternalInput").ap()
    wqk_d = nc.dram_tensor("wqk", [n_ct, P, 2 * C], BF16, kind="ExternalInput").ap()
    wv_d = nc.dram_tensor("wv", [n_ct, P, C], BF16, kind="ExternalInput").ap()
    wp_d = nc.dram_tensor("wp", [N_ET, P, C], BF16, kind="ExternalInput").ap()
    maskT_d = nc.dram_tensor("maskT", [P, P], FP32, kind="ExternalInput").ap()
    out_d = nc.dram_tensor("out", [T, C], BF16, kind="ExternalOutput").ap()
    scr_d = nc.dram_tensor("scr", [N_SUP, 4, TCH], FP32, kind="ExternalOutput").ap()

    with tile.TileContext(nc) as tc:
        with (
            tc.tile_pool(name="persist", bufs=1) as persist,
            tc.tile_pool(name="small", bufs=1) as small,
        ):
            ones = small.tile([P, 1], BF16, name="ones", tag="ones")
            warm_in = small.tile([P, TCH], BF16, name="warm_in", tag="warm_in")
            nc.vector.memset(ones[:], 1.0)
            nc.vector.memset(warm_in[:], 0.0)
            # PE warmup: ~22 wide (512-col) matmuls keep the PE busy from the
            # end of the framework preamble (~7us) until the first input
            # descriptors complete (~12us), ramping the HAM clock to 2.4 GHz.
            # Wide matmuls so the stretch is matmul-bound, not issue-bound.
            with tc.tile_pool(name="warm_ps", bufs=2, space="PSUM") as warm_ps:
                wpss = [
                    warm_ps.tile([1, TCH], FP32, name="wps", tag=f"wps{i}")
                    for i in range(2)
                ]
                for i in range(22):
                    nc.tensor.matmul(
                        wpss[i % 2][:], ones[:], warm_in[:], start=True, stop=True
                    )

            # persistent SBUF arrays
            qT = [persist.tile([P, T], BF16, name=f"qT{e}", tag=f"qT{e}") for e in range(N_ET)]
            kT = [persist.tile([P, T], BF16, name=f"kT{e}", tag=f"kT{e}") for e in range(N_ET)]
            v = [persist.tile([P, C], BF16, name=f"v{t}", tag=f"v{t}") for t in range(N_TT)]
            maskT = small.tile([P, P], FP32, name="maskT", tag="maskT")
            # w_proj persists into phase 2; loaded early on the scalar queue.
            wp = persist.tile([P, N_ET, C], BF16, name="wp", tag="wp")

            # ---------------- phase 1: projections ----------------
            with (
                tc.tile_pool(name="ph1", bufs=1) as ph1,
                tc.tile_pool(name="ph1ps", bufs=8, space="PSUM") as ph1ps,
            ):
                xT = [ph1.tile([P, T], BF16, name=f"xT{c}", tag=f"xT{c}") for c in range(n_ct)]
                wqk = ph1.tile([P, n_ct, 2 * C], BF16, name="wqk", tag="wqk")
                wv = ph1.tile([P, n_ct, C], BF16, name="wv", tag="wv")

                # gpsimd (SWDGE): wqk eb0 (cols 0:512 — everything the opening
                # passes' e-groups 0-3 touch) per-c first, then big blocks.
                # Two late xT h1 halves ride on gpsimd between eb0 and eb1 to
                # balance the three ~90-100 GB/s queues (input load is
                # HBM-BW-bound at ~350 GB/s aggregate).
                for c in range(n_ct):
                    nc.gpsimd.dma_start(wqk[:, c, :TCH], wqk_d[c, :, :TCH])
                h1 = slice(T // 2, T)
                gp_x = [c for c in range(n_ct) if c >= 6]
                for c in gp_x:
                    nc.gpsimd.dma_start(xT[c][:, h1], xT_d[c * P : (c + 1) * P, h1])
                for eb in range(1, 2 * C // TCH):
                    nc.gpsimd.dma_start(
                        wqk[:, :, eb * TCH : (eb + 1) * TCH],
                        wqk_d[:, :, eb * TCH : (eb + 1) * TCH].rearrange(
                            "c p e -> p c e"
                        ),
                    )

                # xT: parity split across sync/scalar HWDGE queues, two
                # 1024-col halves per c-tile; all h0 halves (pass A) first.
                ch = [c for c in range(n_ct) if c % 2 == 0]
                co = [c for c in range(n_ct) if c % 2 == 1]
                for h in range(2):
                    hs = slice(h * (T // 2), (h + 1) * (T // 2))
                    for i in range(max(len(ch), len(co))):
                        if i < len(ch):
                            c = ch[i]
                            if h == 1 and c in gp_x:
                                continue
                            nc.sync.dma_start(
                                xT[c][:, hs], xT_d[c * P : (c + 1) * P, hs]
                            )
                        if i < len(co):
                            c = co[i]
                            if h == 1 and c in gp_x:
                                continue
                            nc.scalar.dma_start(
                                xT[c][:, hs], xT_d[c * P : (c + 1) * P, hs]
                            )

                # wv: halves on sync/scalar after xT; wp on scalar; mask last.
                half = (n_ct + 1) // 2
                nc.sync.dma_start(
                    wv[:, :half, :], wv_d[:half].rearrange("c p e -> p c e")
                )
                nc.scalar.dma_start(
                    wv[:, half:, :], wv_d[half:].rearrange("c p e -> p c e")
                )
                nc.scalar.dma_start(wp[:], wp_d.rearrange("c p e -> p c e"))
                nc.sync.dma_start(maskT[:], maskT_d[:])

                # qT/kT: psum[e-tile, t-chunk] = sum_c w_qk[c, e].T @ xT[c, t]
                # Opening: e-groups 0-3 in two c-OUTER passes over the xT
                # halves (pass A: tc 0-1 needs only each tile's first 1024
                # cols, pass B: tc 2-3).  4 e-groups x 2 chunks = 8 PSUM
                # banks per pass, and each arriving 256KB half feeds 4096
                # PE cycles (~150 GB/s demand, matching 2-queue delivery).
                for h in range(2):
                    pss = [
                        [
                            ph1ps.tile([P, TCH], FP32, name="qkps01", tag="qkps")
                            for _ in range(2)
                        ]
                        for _ in range(4)
                    ]
                    for c in range(n_ct):
                        for e in range(4):
                            for ti in range(2):
                                tc_i = 2 * h + ti
                                nc.tensor.matmul(
                                    pss[e][ti][:],
                                    wqk[:, c, e * P : (e + 1) * P],
                                    xT[c][:, tc_i * TCH : (tc_i + 1) * TCH],
                                    start=(c == 0),
                                    stop=(c == n_ct - 1),
                                )
                    for e in range(4):
                        for ti in range(2):
                            tc_i = 2 * h + ti
                            dst_ap = qT[e][:, tc_i * TCH : (tc_i + 1) * TCH]
                            if (e * 4 + tc_i) % 2 == 0:
                                nc.vector.tensor_copy(dst_ap, pss[e][ti][:])
                            else:
                                nc.scalar.copy(dst_ap, pss[e][ti][:])

                for e in range(4, 2 * N_ET):
                    dst = qT[e] if e < N_ET else kT[e - N_ET]
                    pss = [
                        ph1ps.tile([P, TCH], FP32, name="qkps", tag="qkps")
                        for _ in range(T // TCH)
                    ]
                    for c in range(n_ct):
                        for tc_i in range(T // TCH):
                            nc.tensor.matmul(
                                pss[tc_i][:],
                                wqk[:, c, e * P : (e + 1) * P],
                                xT[c][:, tc_i * TCH : (tc_i + 1) * TCH],
                                start=(c == 0),
                                stop=(c == n_ct - 1),
                            )
                    for tc_i in range(T // TCH):
                        dst_ap = dst[:, tc_i * TCH : (tc_i + 1) * TCH]
                        if (e * 4 + tc_i) % 2 == 0:
                            nc.vector.tensor_copy(dst_ap, pss[tc_i][:])
                        else:
                            nc.scalar.copy(dst_ap, pss[tc_i][:])

                # v: psum[t-tile, c'-chunk] = sum_c xT[c, t].T @ w_v[c, c']
                for t in range(N_TT):
                    pss = [
                        ph1ps.tile([P, TCH], FP32, name="vps", tag="qkps")
                        for _ in range(C // TCH)
                    ]
                    for c in range(n_ct):
                        for cc in range(C // TCH):
                            nc.tensor.matmul(
                                pss[cc][:],
                                xT[c][:, t * P : (t + 1) * P],
                                wv[:, c, cc * TCH : (cc + 1) * TCH],
                                start=(c == 0),
                                stop=(c == n_ct - 1),
                            )
                    for cc in range(C // TCH):
                        dst_ap = v[t][:, cc * TCH : (cc + 1) * TCH]
                        if (t * 2 + cc) % 2 == 0:
                            nc.vector.tensor_copy(dst_ap, pss[cc][:])
                        else:
                            nc.scalar.copy(dst_ap, pss[cc][:])

            # ---------------- phase 2: attention + proj ----------------
            with (
                tc.tile_pool(name="pt_pool", bufs=18) as pt_pool,
                tc.tile_pool(name="ot_pool", bufs=3) as ot_pool,
                tc.tile_pool(name="stage", bufs=3) as stage,
                tc.tile_pool(name="st_ps", bufs=2, space="PSUM") as st_ps,
                tc.tile_pool(name="sums_ps", bufs=1, space="PSUM") as sums_ps,
                tc.tile_pool(name="ot_ps", bufs=2, space="PSUM") as ot_ps,
                tc.tile_pool(name="pr_ps", bufs=3, space="PSUM") as pr_ps,
            ):
                def emit_proj(t0, ot_sb, rt):
                    """proj for the supertile starting at t0, scaled by 1/sums.
                    dch-outer so each 512-wide output chunk's copy+DMA starts
                    while the next chunk's matmuls still run (lean tail)."""
                    for k in range(TCH // P):  # t-tile within supertile
                        for dch in range(C // TCH):
                            prs = pr_ps.tile([P, TCH], FP32, name="pr", tag="pr")
                            for g in range(N_ET):
                                nc.tensor.matmul(
                                    prs[:],
                                    ot_sb[g][:, k * P : (k + 1) * P],
                                    wp[:, g, dch * TCH : (dch + 1) * TCH],
                                    start=(g == 0),
                                    stop=(g == N_ET - 1),
                                )
                            osb_out = stage.tile([P, TCH], BF16, name="osb_out", tag="osb_out")
                            if dch % 2 == 0:
                                nc.scalar.activation(
                                    osb_out[:],
                                    prs[:],
                                    mybir.ActivationFunctionType.Copy,
                                    scale=rt[:, k : k + 1],
                                )
                            else:
                                nc.vector.tensor_scalar_mul(
                                    osb_out[:], prs[:], rt[:, k : k + 1]
                                )
                            nc.sync.dma_start(
                                out_d[
                                    t0 + k * P : t0 + (k + 1) * P,
                                    dch * TCH : (dch + 1) * TCH,
                                ],
                                osb_out[:],
                            )

                pending = None  # (t0, ot_sb, rt) of the previous supertile
                for i in range(N_SUP):  # supertile: t in [i*TCH, (i+1)*TCH)
                    t0 = i * TCH
                    n_st = 4 * i + 4  # causal s-tiles
                    ptiles = []
                    # --- ST + exp per s-tile ---
                    for j in range(n_st):
                        off = max(0, j - 4 * i) * P  # first valid t column
                        st = st_ps.tile([P, TCH], FP32, name="st", tag="st")
                        for e in range(N_ET):
                            nc.tensor.matmul(
                                st[:, off:TCH],
                                kT[e][:, j * P : (j + 1) * P],
                                qT[e][:, t0 + off : t0 + TCH],
                                start=(e == 0),
                                stop=(e == N_ET - 1),
                            )
                        if j >= 4 * i:  # diagonal block: strict-upper (s>t) mask
                            nc.vector.tensor_add(
                                st[:, off : off + P], st[:, off : off + P], maskT[:]
                            )
                        pt = pt_pool.tile([P, TCH], BF16, name="pt", tag="pt")
                        nc.scalar.activation(
                            pt[:, off:TCH],
                            st[:, off:TCH],
                            mybir.ActivationFunctionType.Exp,
                            scale=SCALE,
                        )
                        ptiles.append((pt, off))

                    # --- row sums via ones-matmul (acc over s-tiles) ---
                    # j=0 always has off=0, so the first (start=True) matmul
                    # covers the full width; later partial-width matmuls
                    # accumulate into their column subrange only.
                    sums = sums_ps.tile([1, TCH], FP32, name="sums", tag="sums")
                    for j in range(n_st):
                        pt, off = ptiles[j]
                        nc.tensor.matmul(
                            sums[:, off:TCH],
                            ones[:],
                            pt[:, off:TCH],
                            start=(j == 0),
                            stop=(j == n_st - 1),
                        )
                    srow = stage.tile([1, TCH], FP32, name="srow", tag="srow")
                    nc.vector.tensor_copy(srow[:], sums[:])
                    nc.sync.dma_start(scr_d[i, 0:1, :], srow[:])
                    rt0 = stage.tile([P, N_SUP], FP32, name="rt0", tag="rt0")
                    nc.sync.dma_start(
                        rt0[:], scr_d[i, 0].rearrange("(f q) -> q f", q=P)
                    )
                    rt = stage.tile([P, N_SUP], FP32, name="rt", tag="rt")
                    nc.vector.reciprocal(rt[:], rt0[:])

                    # --- previous supertile's proj (hides the recip roundtrip) ---
                    if pending is not None:
                        emit_proj(*pending)

                    # --- OT[c'-tile, t-chunk] = sum_s v[s,c'].T @ P~T[s,t] ---
                    ot_sb = []
                    for g in range(N_ET):
                        ot = ot_ps.tile([P, TCH], FP32, name="ot", tag="ot")
                        for j in range(n_st):
                            pt, off = ptiles[j]
                            nc.tensor.matmul(
                                ot[:, off:TCH],
                                v[j][:, g * P : (g + 1) * P],
                                pt[:, off:TCH],
                                start=(j == 0),
                                stop=(j == n_st - 1),
                            )
                        osb = ot_pool.tile([P, TCH], BF16, name="osb", tag=f"osb{g % 3}")
                        nc.vector.tensor_copy(osb[:], ot[:])
                        ot_sb.append(osb)

                    pending = (t0, ot_sb, rt)

                emit_proj(*pending)

    nc.compile()
    return nc


def kernel(x, w_attn, b_attn, w_proj, b_proj):
    x = np.asarray(x, dtype=np.float32)
    w_attn = np.asarray(w_attn, dtype=np.float32)
    b_attn = np.asarray(b_attn, dtype=np.float32)
    w_proj = np.asarray(w_proj, dtype=np.float32)
    b_proj = np.asarray(b_proj, dtype=np.float32)
    assert x.shape == (B, T, C)

    aug = bool(np.any(b_attn != 0.0))
    n_ct = C // P + (1 if aug else 0)
    if n_ct not in _cache:
        _cache[n_ct] = _build(n_ct)
    nc = _cache[n_ct]

    bf = ml_dtypes.bfloat16
    if aug:
        wqk = np.zeros((n_ct, P, 2 * C), dtype=bf)
        wqk.reshape(n_ct * P, 2 * C)[:C] = w_attn[:, : 2 * C].astype(bf)
        wqk.reshape(n_ct * P, 2 * C)[C] = b_attn[: 2 * C].astype(bf)
        wv = np.zeros((n_ct, P, C), dtype=bf)
        wv.reshape(n_ct * P, C)[:C] = w_attn[:, 2 * C :].astype(bf)
        wv.reshape(n_ct * P, C)[C] = b_attn[2 * C :].astype(bf)
    else:
        wqk = np.ascontiguousarray(w_attn[:, : 2 * C]).astype(bf).reshape(n_ct, P, 2 * C)
        wv = np.ascontiguousarray(w_attn[:, 2 * C :]).astype(bf).reshape(n_ct, P, C)
    wp = w_proj.astype(bf).reshape(N_ET, P, C)

    # strict upper triangle (s > t) additive mask for transposed [s, t] blocks
    maskT = np.where(
        np.arange(P)[:, None] > np.arange(P)[None, :], np.float32(NEG), np.float32(0.0)
    ).astype(np.float32)

    in_maps = []
    for b in range(B):
        xT = np.ascontiguousarray(x[b].T).astype(bf)
        if aug:
            xTa = np.zeros((n_ct * P, T), dtype=bf)
            xTa[:C] = xT
            xTa[C] = bf(1.0)
            xT = xTa
        in_maps.append({"xT": xT, "wqk": wqk, "wv": wv, "wp": wp, "maskT": maskT})

    global _last_in_maps
    _last_in_maps = in_maps
    res = run_bass_kernel_spmd(nc, in_maps, core_ids=list(range(8)))
    out = np.stack([res.results[b]["out"] for b in range(B)]).astype(np.float32)
    if np.any(b_proj != 0.0):
        out = out + b_proj[None, None, :]
    return out


if __name__ == "__main__":
    rng = np.random.default_rng(0)
    x = rng.standard_normal((B, T, C), dtype=np.float32)
    w_attn = rng.standard_normal((C, 3 * C), dtype=np.float32) / np.sqrt(C)
    b_attn = np.zeros(3 * C, dtype=np.float32)
    w_proj = rng.standard_normal((C, C), dtype=np.float32) / np.sqrt(C)
    b_proj = np.zeros(C, dtype=np.float32)
    out = kernel(x, w_attn, b_attn, w_proj, b_proj)
    print(out.shape, out.dtype)


# revision 13
# speedup vs baseline: 1.0401x; 1.0109x over previous
"""Causal self-attention kernel for Trainium2 (8 NeuronCores, data-parallel).

Problem: B=8, T=2048, C=1024 single-head causal attention:
    qkv = x @ w_attn + b_attn ; q,k,v = split(qkv)
    attn = softmax(q @ k.T / sqrt(C) + causal_mask)
    out  = (attn @ v) @ w_proj + b_proj

Sharding: pure data parallel — one batch element per core, weights replicated,
no collectives.

Per-core algorithm (all matmuls bf16 operands, fp32 PSUM accumulate):
  host: xT = x[b].T cast bf16 (so the contraction dim is the partition dim
        everywhere on device; no on-device transposes needed anywhere).
  ph1:  qT[e,t], kT[e,s]  <- matmul(lhsT=w_qk[c,e-tile], rhs=xT[c,t])   [e,t] layout
        v[t,c']           <- matmul(lhsT=xT[c,t-tile],  rhs=w_v[c,c'])  natural layout
  ph2:  per 512-wide t-chunk ("supertile"), per 128-wide s-tile (causal only):
        ST[s,t]  <- matmul(lhsT=kT[e,s-tile], rhs=qT[e,t-chunk])  (8 e-tiles acc)
        P~T[s,t] <- exp(ST/sqrt(C) + mask)   (no max-subtract; logits are O(1))
        sums[t]  <- matmul(lhsT=ones[s,1], rhs=P~T)  (acc over s-tiles)
        OT[c',t] <- matmul(lhsT=v[s-tile,c'-tile], rhs=P~T[s-tile,t-chunk])
        out[t,d] <- matmul(lhsT=OT[c',t-tile], rhs=w_proj[c',d]) * (1/sums[t])
  The 1/sums normalization is folded into the final PSUM->SBUF copy as a
  per-partition activation scale (everything between exp and out is linear).
  The proj stage runs one supertile behind (software pipeline) so the
  sums->reciprocal DMA roundtrip never stalls the PE.

DMA plan (3 queues: sync + scalar are HWDGE, gpsimd is SWDGE):
  - xT c-tiles split by parity across sync/scalar, each tile DMA'd as two
    1024-col halves so the first matmuls can start ~4us in.
  - weights live in single 3-dim SBUF tiles ([P, c, cols]) so each eb-block
    is ONE big descriptor (per-descriptor issue is ~0.65us on a queue;
    32 small descriptors would serialize for 20us+).
  - wqk eb0 goes per-c-tile first (matches the c-outer e-group 0-1 ramp
    ordering), the rest as 1MB descriptors.
  - No PE warmup: the Tensor queue's framework preamble ends ~6.5us anyway,
    by which time the first xT/wqk blocks have landed; the p-state clock
    ramps during the DMA-limited opening instead of on dummy matmuls.

Output is written bf16 (halves the out DMA) and upcast on host.

b_attn is folded in by augmenting x with a ones column (padded to a full
128-partition tile) only when it is nonzero; b_proj is added on the host.
"""

import sys

if "/opt/trn_rl_repo" not in sys.path:
    sys.path.insert(0, "/opt/trn_rl_repo")

import numpy as np
import ml_dtypes

import concourse.bacc as bacc
import concourse.mybir as mybir
import concourse.tile as tile
from concourse.bass_utils import run_bass_kernel_spmd

B, T, C = 8, 2048, 1024
P = 128  # partitions
TCH = 512  # t-chunk (moving free dim)
N_TT = T // P  # 16 t-tiles
N_SUP = T // TCH  # 4 supertiles
N_ET = C // P  # 8 e-tiles (q/k feature dim)
SCALE = 1.0 / float(np.sqrt(np.float32(C)))
NEG = -10000000000.0

BF16 = mybir.dt.bfloat16
FP32 = mybir.dt.float32

_cache = {}


def _build(n_ct):
    """Build the SPMD Bass program. n_ct = number of 128-wide c-tiles of the
    (possibly ones-augmented) input feature dim."""
    nc = bacc.Bacc("TRN2", target_bir_lowering=False, debug=False, num_devices=8)

    xT_d = nc.dram_tensor("xT", [n_ct * P, T], BF16, kind="ExternalInput").ap()
    wqk_d = nc.dram_tensor("wqk", [n_ct, P, 2 * C], BF16, kind="ExternalInput").ap()
    wv_d = nc.dram_tensor("wv", [n_ct, P, C], BF16, kind="ExternalInput").ap()
    wp_d = nc.dram_tensor("wp", [N_ET, P, C], BF16, kind="ExternalInput").ap()
    maskT_d = nc.dram_tensor("maskT", [P, P], FP32, kind="ExternalInput").ap()
    out_d = nc.dram_tensor("out", [T, C], BF16, kind="ExternalOutput").ap()
    scr_d = nc.dram_tensor("scr", [N_SUP, 4, TCH], FP32, kind="ExternalOutput").ap()

    with tile.TileContext(nc) as tc:
        with (
            tc.tile_pool(name="persist", bufs=1) as persist,
            tc.tile_pool(name="small", bufs=1) as small,
        ):
            ones = small.tile([P, 1], BF16, name="ones", tag="ones")
            warm_in = small.tile([P, TCH], BF16, name="warm_in", tag="warm_in")
            nc.vector.memset(ones[:], 1.0)
            nc.vector.memset(warm_in[:], 0.0)
            # PE warmup: ~22 wide (512-col) matmuls keep the PE busy from the
            # end of the framework preamble (~7us) until the first input
            # descriptors complete (~12us), ramping the HAM clock to 2.4 GHz.
            # Wide matmuls so the stretch is matmul-bound, not issue-bound.
            with tc.tile_pool(name="warm_ps", bufs=1, space="PSUM") as warm_ps:
                wps = warm_ps.tile([1, TCH], FP32, name="wps", tag="wps")
                NWARM = 15
                for i in range(NWARM):
                    nc.tensor.matmul(
                        wps[:], ones[:], warm_in[:],
                        start=(i == 0), stop=(i == NWARM - 1),
                    )

            # persistent SBUF arrays
            qT = [persist.tile([P, T], BF16, name=f"qT{e}", tag=f"qT{e}") for e in range(N_ET)]
            kT = [persist.tile([P, T], BF16, name=f"kT{e}", tag=f"kT{e}") for e in range(N_ET)]
            v = [persist.tile([P, C], BF16, name=f"v{t}", tag=f"v{t}") for t in range(N_TT)]
            maskT = small.tile([P, P], FP32, name="maskT", tag="maskT")
            # w_proj persists into phase 2; loaded early on the scalar queue.
            wp = persist.tile([P, N_ET, C], BF16, name="wp", tag="wp")

            # ---------------- phase 1: projections ----------------
            with (
                tc.tile_pool(name="ph1", bufs=1) as ph1,
                tc.tile_pool(name="ph1ps", bufs=8, space="PSUM") as ph1ps,
            ):
                xT = [ph1.tile([P, T], BF16, name=f"xT{c}", tag=f"xT{c}") for c in range(n_ct)]
                wqk = ph1.tile([P, n_ct, 2 * C], BF16, name="wqk", tag="wqk")
                wv = ph1.tile([P, n_ct, C], BF16, name="wv", tag="wv")

                # gpsimd (SWDGE): wqk eb0 (cols 0:512 — everything the opening
                # passes' e-groups 0-3 touch) per-c first, then big blocks.
                # Two late xT h1 halves ride on gpsimd between eb0 and eb1 to
                # balance the three ~90-100 GB/s queues (input load is
                # HBM-BW-bound at ~350 GB/s aggregate).
                # ladder: c0's block split so the very first matmuls unblock
                # one descriptor-latency earlier
                nc.gpsimd.dma_start(wqk[:, 0, :256], wqk_d[0, :, :256])
                nc.gpsimd.dma_start(wqk[:, 0, 256:TCH], wqk_d[0, :, 256:TCH])
                for c in range(1, n_ct):
                    nc.gpsimd.dma_start(wqk[:, c, :TCH], wqk_d[c, :, :TCH])
                h1 = slice(T // 2, T)
                gp_x = [c for c in range(n_ct) if c >= 6]
                for c in gp_x:
                    nc.gpsimd.dma_start(xT[c][:, h1], xT_d[c * P : (c + 1) * P, h1])
                for eb in range(1, 2 * C // TCH):
                    nc.gpsimd.dma_start(
                        wqk[:, :, eb * TCH : (eb + 1) * TCH],
                        wqk_d[:, :, eb * TCH : (eb + 1) * TCH].rearrange(
                            "c p e -> p c e"
                        ),
                    )

                # xT: parity split across sync/scalar HWDGE queues, two
                # 1024-col halves per c-tile; all h0 halves (pass A) first.
                ch = [c for c in range(n_ct) if c % 2 == 0]
                co = [c for c in range(n_ct) if c % 2 == 1]
                # ladder: first halves of c0/c1 as two 512-col descriptors
                nc.sync.dma_start(xT[0][:, :TCH], xT_d[0:P, :TCH])
                nc.sync.dma_start(xT[0][:, TCH : T // 2], xT_d[0:P, TCH : T // 2])
                if n_ct > 1:
                    nc.scalar.dma_start(xT[1][:, :TCH], xT_d[P : 2 * P, :TCH])
                    nc.scalar.dma_start(
                        xT[1][:, TCH : T // 2], xT_d[P : 2 * P, TCH : T // 2]
                    )
                for h in range(2):
                    hs = slice(h * (T // 2), (h + 1) * (T // 2))
                    for i in range(max(len(ch), len(co))):
                        if i < len(ch):
                            c = ch[i]
                            if (h == 1 and c in gp_x) or (h == 0 and c == 0):
                                pass
                            else:
                                nc.sync.dma_start(
                                    xT[c][:, hs], xT_d[c * P : (c + 1) * P, hs]
                                )
                        if i < len(co):
                            c = co[i]
                            if (h == 1 and c in gp_x) or (h == 0 and c == 1):
                                pass
                            else:
                                nc.scalar.dma_start(
                                    xT[c][:, hs], xT_d[c * P : (c + 1) * P, hs]
                                )

                # wv: halves on sync/scalar after xT; wp on scalar; mask last.
                half = (n_ct + 1) // 2
                nc.sync.dma_start(
                    wv[:, :half, :], wv_d[:half].rearrange("c p e -> p c e")
                )
                nc.scalar.dma_start(
                    wv[:, half:, :], wv_d[half:].rearrange("c p e -> p c e")
                )
                nc.scalar.dma_start(wp[:], wp_d.rearrange("c p e -> p c e"))
                nc.sync.dma_start(maskT[:], maskT_d[:])

                # qT/kT: psum[e-tile, t-chunk] = sum_c w_qk[c, e].T @ xT[c, t]
                # Opening: e-groups 0-3 in two c-OUTER passes over the xT
                # halves (pass A: tc 0-1 needs only each tile's first 1024
                # cols, pass B: tc 2-3).  4 e-groups x 2 chunks = 8 PSUM
                # banks per pass, and each arriving 256KB half feeds 4096
                # PE cycles (~150 GB/s demand, matching 2-queue delivery).
                for h in range(2):
                    pss = [
                        [
                            ph1ps.tile([P, TCH], FP32, name="qkps01", tag="qkps")
                            for _ in range(2)
                        ]
                        for _ in range(4)
                    ]
                    for c in range(n_ct):
                        for e in range(4):
                            for ti in range(2):
                                tc_i = 2 * h + ti
                                nc.tensor.matmul(
                                    pss[e][ti][:],
                                    wqk[:, c, e * P : (e + 1) * P],
                                    xT[c][:, tc_i * TCH : (tc_i + 1) * TCH],
                                    start=(c == 0),
                                    stop=(c == n_ct - 1),
                                )
                    for e in range(4):
                        for ti in range(2):
                            tc_i = 2 * h + ti
                            dst_ap = qT[e][:, tc_i * TCH : (tc_i + 1) * TCH]
                            if (e * 4 + tc_i) % 2 == 0:
                                nc.vector.tensor_copy(dst_ap, pss[e][ti][:])
                            else:
                                nc.scalar.copy(dst_ap, pss[e][ti][:])

                for e in range(4, 2 * N_ET):
                    dst = qT[e] if e < N_ET else kT[e - N_ET]
                    pss = [
                        ph1ps.tile([P, TCH], FP32, name="qkps", tag="qkps")
                        for _ in range(T // TCH)
                    ]
                    for c in range(n_ct):
                        for tc_i in range(T // TCH):
                            nc.tensor.matmul(
                                pss[tc_i][:],
                                wqk[:, c, e * P : (e + 1) * P],
                                xT[c][:, tc_i * TCH : (tc_i + 1) * TCH],
                                start=(c == 0),
                                stop=(c == n_ct - 1),
                            )
                    for tc_i in range(T // TCH):
                        dst_ap = dst[:, tc_i * TCH : (tc_i + 1) * TCH]
                        if (e * 4 + tc_i) % 2 == 0:
                            nc.vector.tensor_copy(dst_ap, pss[tc_i][:])
                        else:
                            nc.scalar.copy(dst_ap, pss[tc_i][:])

                # v: psum[t-tile, c'-chunk] = sum_c xT[c, t].T @ w_v[c, c']
                for t in range(N_TT):
                    pss = [
                        ph1ps.tile([P, TCH], FP32, name="vps", tag="qkps")
                        for _ in range(C // TCH)
                    ]
                    for c in range(n_ct):
                        for cc in range(C // TCH):
                            nc.tensor.matmul(
                                pss[cc][:],
                                xT[c][:, t * P : (t + 1) * P],
                                wv[:, c, cc * TCH : (cc + 1) * TCH],
                                start=(c == 0),
                                stop=(c == n_ct - 1),
                            )
                    for cc in range(C // TCH):
                        dst_ap = v[t][:, cc * TCH : (cc + 1) * TCH]
                        if (t * 2 + cc) % 2 == 0:
                            nc.vector.tensor_copy(dst_ap, pss[cc][:])
                        else:
                            nc.scalar.copy(dst_ap, pss[cc][:])

            # ---------------- phase 2: attention + proj ----------------
            with (
                tc.tile_pool(name="pt_pool", bufs=18) as pt_pool,
                tc.tile_pool(name="ot_pool", bufs=3) as ot_pool,
                tc.tile_pool(name="stage", bufs=3) as stage,
                tc.tile_pool(name="st_ps", bufs=2, space="PSUM") as st_ps,
                tc.tile_pool(name="sums_ps", bufs=1, space="PSUM") as sums_ps,
                tc.tile_pool(name="ot_ps", bufs=2, space="PSUM") as ot_ps,
                tc.tile_pool(name="pr_ps", bufs=3, space="PSUM") as pr_ps,
            ):
                def emit_proj(t0, ot_sb, rt):
                    """proj for the supertile starting at t0, scaled by 1/sums.
                    dch-outer so each 512-wide output chunk's copy+DMA starts
                    while the next chunk's matmuls still run (lean tail)."""
                    for k in range(TCH // P):  # t-tile within supertile
                        for dch in range(C // TCH):
                            prs = pr_ps.tile([P, TCH], FP32, name="pr", tag="pr")
                            for g in range(N_ET):
                                nc.tensor.matmul(
                                    prs[:],
                                    ot_sb[g][:, k * P : (k + 1) * P],
                                    wp[:, g, dch * TCH : (dch + 1) * TCH],
                                    start=(g == 0),
                                    stop=(g == N_ET - 1),
                                )
                            osb_out = stage.tile([P, TCH], BF16, name="osb_out", tag="osb_out")
                            if dch % 2 == 0:
                                nc.scalar.activation(
                                    osb_out[:],
                                    prs[:],
                                    mybir.ActivationFunctionType.Copy,
                                    scale=rt[:, k : k + 1],
                                )
                            else:
                                nc.vector.tensor_scalar_mul(
                                    osb_out[:], prs[:], rt[:, k : k + 1]
                                )
                            dq = nc.sync if dch % 2 == 0 else nc.gpsimd
                            dq.dma_start(
                                out_d[
                                    t0 + k * P : t0 + (k + 1) * P,
                                    dch * TCH : (dch + 1) * TCH,
                                ],
                                osb_out[:],
                            )

                pending = None  # (t0, ot_sb, rt) of the previous supertile
                for i in range(N_SUP):  # supertile: t in [i*TCH, (i+1)*TCH)
                    t0 = i * TCH
                    n_st = 4 * i + 4  # causal s-tiles
                    ptiles = []
                    # fp32 running sum of the P~T tiles, built on the (idle)
                    # vector engine in the shadow of the ST matmuls; replaces
                    # the per-s-tile ones-matmuls (17.4k PE cycles total with
                    # a single 512-cycle matmul per supertile).
                    sacc = stage.tile([P, TCH], FP32, name="sacc", tag="sacc")
                    # --- ST + exp per s-tile ---
                    for j in range(n_st):
                        off = max(0, j - 4 * i) * P  # first valid t column
                        st = st_ps.tile([P, TCH], FP32, name="st", tag="st")
                        for e in range(N_ET):
                            nc.tensor.matmul(
                                st[:, off:TCH],
                                kT[e][:, j * P : (j + 1) * P],
                                qT[e][:, t0 + off : t0 + TCH],
                                start=(e == 0),
                                stop=(e == N_ET - 1),
                            )
                        if j >= 4 * i:  # diagonal block: strict-upper (s>t) mask
                            nc.vector.tensor_add(
                                st[:, off : off + P], st[:, off : off + P], maskT[:]
                            )
                        pt = pt_pool.tile([P, TCH], BF16, name="pt", tag="pt")
                        nc.scalar.activation(
                            pt[:, off:TCH],
                            st[:, off:TCH],
                            mybir.ActivationFunctionType.Exp,
                            scale=SCALE,
                        )
                        if j == 0:  # j=0 has off=0: full-width init
                            nc.vector.tensor_copy(sacc[:], pt[:])
                        else:
                            nc.vector.tensor_add(
                                sacc[:, off:TCH], sacc[:, off:TCH], pt[:, off:TCH]
                            )
                        ptiles.append((pt, off))

                    # --- previous supertile's proj (hides the recip roundtrip) ---
                    if pending is not None:
                        emit_proj(*pending)

                    # --- OT[c'-tile, t-chunk] = sum_s v[s,c'].T @ P~T[s,t] ---
                    ot_sb = []
                    for g in range(N_ET):
                        ot = ot_ps.tile([P, TCH], FP32, name="ot", tag="ot")
                        for j in range(n_st):
                            pt, off = ptiles[j]
                            nc.tensor.matmul(
                                ot[:, off:TCH],
                                v[j][:, g * P : (g + 1) * P],
                                pt[:, off:TCH],
                                start=(j == 0),
                                stop=(j == n_st - 1),
                            )
                        osb = ot_pool.tile([P, TCH], BF16, name="osb", tag=f"osb{g % 3}")
                        nc.vector.tensor_copy(osb[:], ot[:])
                        ot_sb.append(osb)

                    # --- row sums: one 128->1 ones-matmul over the bf16 cast
                    # of sacc, then the DMA-roundtrip reciprocal.  Emitted
                    # after OT so the PE never waits on the vector chain, and
                    # rt is only needed by proj(i) one supertile later.
                    sacc_bf = stage.tile([P, TCH], BF16, name="sacc_bf", tag="sacc_bf")
                    nc.vector.tensor_copy(sacc_bf[:], sacc[:])
                    sums = sums_ps.tile([1, TCH], FP32, name="sums", tag="sums")
                    nc.tensor.matmul(sums[:], ones[:], sacc_bf[:], start=True, stop=True)
                    srow = stage.tile([1, TCH], FP32, name="srow", tag="srow")
                    nc.vector.tensor_copy(srow[:], sums[:])
                    nc.sync.dma_start(scr_d[i, 0:1, :], srow[:])
                    rt0 = stage.tile([P, N_SUP], FP32, name="rt0", tag="rt0")
                    nc.sync.dma_start(
                        rt0[:], scr_d[i, 0].rearrange("(f q) -> q f", q=P)
                    )
                    rt = stage.tile([P, N_SUP], FP32, name="rt", tag="rt")
                    nc.vector.reciprocal(rt[:], rt0[:])

                    pending = (t0, ot_sb, rt)

                emit_proj(*pending)

    nc.compile()
    return nc


def kernel(x, w_attn, b_attn, w_proj, b_proj):
    x = np.asarray(x, dtype=np.float32)
    w_attn = np.asarray(w_attn, dtype=np.float32)
    b_attn = np.asarray(b_attn, dtype=np.float32)
    w_proj = np.asarray(w_proj, dtype=np.float32)
    b_proj = np.asarray(b_proj, dtype=np.float32)
    assert x.shape == (B, T, C)

    aug = bool(np.any(b_attn != 0.0))
    n_ct = C // P + (1 if aug else 0)
    if n_ct not in _cache:
        _cache[n_ct] = _build(n_ct)
    nc = _cache[n_ct]

    bf = ml_dtypes.bfloat16
    if aug:
        wqk = np.zeros((n_ct, P, 2 * C), dtype=bf)
        wqk.reshape(n_ct * P, 2 * C)[:C] = w_attn[:, : 2 * C].astype(bf)
        wqk.reshape(n_ct * P, 2 * C)[C] = b_attn[: 2 * C].astype(bf)
        wv = np.zeros((n_ct, P, C), dtype=bf)
        wv.reshape(n_ct * P, C)[:C] = w_attn[:, 2 * C :].astype(bf)
        wv.reshape(n_ct * P, C)[C] = b_attn[2 * C :].astype(bf)
    else:
        wqk = np.ascontiguousarray(w_attn[:, : 2 * C]).astype(bf).reshape(n_ct, P, 2 * C)
        wv = np.ascontiguousarray(w_attn[:, 2 * C :]).astype(bf).reshape(n_ct, P, C)
    wp = w_proj.astype(bf).reshape(N_ET, P, C)

    # strict upper triangle (s > t) additive mask for transposed [s, t] blocks
    maskT = np.where(
        np.arange(P)[:, None] > np.arange(P)[None, :], np.float32(NEG), np.float32(0.0)
    ).astype(np.float32)

    in_maps = []
    for b in range(B):
        xT = np.ascontiguousarray(x[b].T).astype(bf)
        if aug:
            xTa = np.zeros((n_ct * P, T), dtype=bf)
            xTa[:C] = xT
            xTa[C] = bf(1.0)
            xT = xTa
        in_maps.append({"xT": xT, "wqk": wqk, "wv": wv, "wp": wp, "maskT": maskT})

    global _last_in_maps
    _last_in_maps = in_maps
    res = run_bass_kernel_spmd(nc, in_maps, core_ids=list(range(8)))
    out = np.stack([res.results[b]["out"] for b in range(B)]).astype(np.float32)
    if np.any(b_proj != 0.0):
        out = out + b_proj[None, None, :]
    return out


if __name__ == "__main__":
    rng = np.random.default_rng(0)
    x = rng.standard_normal((B, T, C), dtype=np.float32)
    w_attn = rng.standard_normal((C, 3 * C), dtype=np.float32) / np.sqrt(C)
    b_attn = np.zeros(3 * C, dtype=np.float32)
    w_proj = rng.standard_normal((C, C), dtype=np.float32) / np.sqrt(C)
    b_proj = np.zeros(C, dtype=np.float32)
    out = kernel(x, w_attn, b_attn, w_proj, b_proj)
    print(out.shape, out.dtype)


# revision 16
# speedup vs baseline: 1.0512x; 1.0106x over previous
"""Causal self-attention kernel for Trainium2 (8 NeuronCores, data-parallel).

Problem: B=8, T=2048, C=1024 single-head causal attention:
    qkv = x @ w_attn + b_attn ; q,k,v = split(qkv)
    attn = softmax(q @ k.T / sqrt(C) + causal_mask)
    out  = (attn @ v) @ w_proj + b_proj

Sharding: pure data parallel — one batch element per core, weights replicated,
no collectives.

Per-core algorithm (all matmuls bf16 operands, fp32 PSUM accumulate):
  host: xT = x[b].T cast bf16 (so the contraction dim is the partition dim
        everywhere on device; no on-device transposes needed anywhere).
  ph1:  qT[e,t], kT[e,s]  <- matmul(lhsT=w_qk[c,e-tile], rhs=xT[c,t])   [e,t] layout
        v[t,c']           <- matmul(lhsT=xT[c,t-tile],  rhs=w_v[c,c'])  natural layout
  ph2:  per 512-wide t-chunk ("supertile"), per 128-wide s-tile (causal only):
        ST[s,t]  <- matmul(lhsT=kT[e,s-tile], rhs=qT[e,t-chunk])  (8 e-tiles acc)
        P~T[s,t] <- exp(ST/sqrt(C) + mask)   (no max-subtract; logits are O(1))
        sums[t]  <- matmul(lhsT=ones[s,1], rhs=P~T)  (acc over s-tiles)
        OT[c',t] <- matmul(lhsT=v[s-tile,c'-tile], rhs=P~T[s-tile,t-chunk])
        out[t,d] <- matmul(lhsT=OT[c',t-tile], rhs=w_proj[c',d]) * (1/sums[t])
  The 1/sums normalization is folded into the final PSUM->SBUF copy as a
  per-partition activation scale (everything between exp and out is linear).
  The proj stage runs one supertile behind (software pipeline) so the
  sums->reciprocal DMA roundtrip never stalls the PE.

DMA plan (3 queues: sync + scalar are HWDGE, gpsimd is SWDGE):
  - xT c-tiles split by parity across sync/scalar, each tile DMA'd as two
    1024-col halves so the first matmuls can start ~4us in.
  - weights live in single 3-dim SBUF tiles ([P, c, cols]) so each eb-block
    is ONE big descriptor (per-descriptor issue is ~0.65us on a queue;
    32 small descriptors would serialize for 20us+).
  - wqk eb0 goes per-c-tile first (matches the c-outer e-group 0-1 ramp
    ordering), the rest as 1MB descriptors.
  - No PE warmup: the Tensor queue's framework preamble ends ~6.5us anyway,
    by which time the first xT/wqk blocks have landed; the p-state clock
    ramps during the DMA-limited opening instead of on dummy matmuls.

Output is written bf16 (halves the out DMA) and upcast on host.

b_attn is folded in by augmenting x with a ones column (padded to a full
128-partition tile) only when it is nonzero; b_proj is added on the host.
"""

import sys

if "/opt/trn_rl_repo" not in sys.path:
    sys.path.insert(0, "/opt/trn_rl_repo")

import numpy as np
import ml_dtypes

import concourse.bacc as bacc
import concourse.mybir as mybir
import concourse.tile as tile
from concourse.bass_utils import run_bass_kernel_spmd

B, T, C = 8, 2048, 1024
P = 128  # partitions
TCH = 512  # t-chunk (moving free dim)
N_TT = T // P  # 16 t-tiles
N_SUP = T // TCH  # 4 supertiles
N_ET = C // P  # 8 e-tiles (q/k feature dim)
SCALE = 1.0 / float(np.sqrt(np.float32(C)))
NEG = -10000000000.0

BF16 = mybir.dt.bfloat16
FP32 = mybir.dt.float32

_cache = {}


def _build(n_ct):
    """Build the SPMD Bass program. n_ct = number of 128-wide c-tiles of the
    (possibly ones-augmented) input feature dim."""
    nc = bacc.Bacc("TRN2", target_bir_lowering=False, debug=False, num_devices=8)

    xT_d = nc.dram_tensor("xT", [n_ct * P, T], BF16, kind="ExternalInput").ap()
    wqk_d = nc.dram_tensor("wqk", [n_ct, P, 2 * C], BF16, kind="ExternalInput").ap()
    wv_d = nc.dram_tensor("wv", [n_ct, P, C], BF16, kind="ExternalInput").ap()
    wp_d = nc.dram_tensor("wp", [N_ET, P, C], BF16, kind="ExternalInput").ap()
    maskT_d = nc.dram_tensor("maskT", [P, P], FP32, kind="ExternalInput").ap()
    out_d = nc.dram_tensor("out", [T, C], BF16, kind="ExternalOutput").ap()
    scr_d = nc.dram_tensor("scr", [N_SUP, 4, TCH], FP32, kind="ExternalOutput").ap()

    with tile.TileContext(nc) as tc:
        with (
            tc.tile_pool(name="persist", bufs=1) as persist,
            tc.tile_pool(name="small", bufs=1) as small,
        ):
            ones = small.tile([P, 1], BF16, name="ones", tag="ones")
            warm_in = small.tile([P, TCH], BF16, name="warm_in", tag="warm_in")
            nc.vector.memset(ones[:], 1.0)
            nc.vector.memset(warm_in[:], 0.0)
            # PE warmup: ~22 wide (512-col) matmuls keep the PE busy from the
            # end of the framework preamble (~7us) until the first input
            # descriptors complete (~12us), ramping the HAM clock to 2.4 GHz.
            # Wide matmuls so the stretch is matmul-bound, not issue-bound.
            with tc.tile_pool(name="warm_ps", bufs=1, space="PSUM") as warm_ps:
                wps = warm_ps.tile([1, TCH], FP32, name="wps", tag="wps")
                NWARM = 15
                for i in range(NWARM):
                    nc.tensor.matmul(
                        wps[:], ones[:], warm_in[:],
                        start=(i == 0), stop=(i == NWARM - 1),
                    )

            # persistent SBUF arrays
            qT = [persist.tile([P, T], BF16, name=f"qT{e}", tag=f"qT{e}") for e in range(N_ET)]
            kT = [persist.tile([P, T], BF16, name=f"kT{e}", tag=f"kT{e}") for e in range(N_ET)]
            v = [persist.tile([P, C], BF16, name=f"v{t}", tag=f"v{t}") for t in range(N_TT)]
            maskT = small.tile([P, P], FP32, name="maskT", tag="maskT")
            # w_proj persists into phase 2; loaded early on the scalar queue.
            wp = persist.tile([P, N_ET, C], BF16, name="wp", tag="wp")

            # ---------------- phase 1: projections ----------------
            with (
                tc.tile_pool(name="ph1", bufs=1) as ph1,
                tc.tile_pool(name="ph1ps", bufs=8, space="PSUM") as ph1ps,
            ):
                xT = [ph1.tile([P, T], BF16, name=f"xT{c}", tag=f"xT{c}") for c in range(n_ct)]
                wqk = ph1.tile([P, n_ct, 2 * C], BF16, name="wqk", tag="wqk")
                wv = ph1.tile([P, n_ct, C], BF16, name="wv", tag="wv")

                # gpsimd (SWDGE): wqk eb0 (cols 0:512 — everything the opening
                # passes' e-groups 0-3 touch) per-c first, then big blocks.
                # Two late xT h1 halves ride on gpsimd between eb0 and eb1 to
                # balance the three ~90-100 GB/s queues (input load is
                # HBM-BW-bound at ~350 GB/s aggregate).
                # ladder: c0's block split so the very first matmuls unblock
                # one descriptor-latency earlier
                nc.gpsimd.dma_start(wqk[:, 0, :256], wqk_d[0, :, :256])
                nc.gpsimd.dma_start(wqk[:, 0, 256:TCH], wqk_d[0, :, 256:TCH])
                for c in range(1, n_ct):
                    nc.gpsimd.dma_start(wqk[:, c, :TCH], wqk_d[c, :, :TCH])
                h1 = slice(T // 2, T)
                gp_x = [c for c in range(n_ct) if c >= 6]
                for c in gp_x:
                    nc.gpsimd.dma_start(xT[c][:, h1], xT_d[c * P : (c + 1) * P, h1])
                for eb in range(1, 2 * C // TCH):
                    nc.gpsimd.dma_start(
                        wqk[:, :, eb * TCH : (eb + 1) * TCH],
                        wqk_d[:, :, eb * TCH : (eb + 1) * TCH].rearrange(
                            "c p e -> p c e"
                        ),
                    )

                # xT: parity split across sync/scalar HWDGE queues, two
                # 1024-col halves per c-tile; all h0 halves (pass A) first.
                ch = [c for c in range(n_ct) if c % 2 == 0]
                co = [c for c in range(n_ct) if c % 2 == 1]
                # ladder: first halves of c0/c1 as two 512-col descriptors
                nc.sync.dma_start(xT[0][:, :TCH], xT_d[0:P, :TCH])
                nc.sync.dma_start(xT[0][:, TCH : T // 2], xT_d[0:P, TCH : T // 2])
                if n_ct > 1:
                    nc.scalar.dma_start(xT[1][:, :TCH], xT_d[P : 2 * P, :TCH])
                    nc.scalar.dma_start(
                        xT[1][:, TCH : T // 2], xT_d[P : 2 * P, TCH : T // 2]
                    )
                for h in range(2):
                    hs = slice(h * (T // 2), (h + 1) * (T // 2))
                    for i in range(max(len(ch), len(co))):
                        if i < len(ch):
                            c = ch[i]
                            if (h == 1 and c in gp_x) or (h == 0 and c == 0):
                                pass
                            else:
                                nc.sync.dma_start(
                                    xT[c][:, hs], xT_d[c * P : (c + 1) * P, hs]
                                )
                        if i < len(co):
                            c = co[i]
                            if (h == 1 and c in gp_x) or (h == 0 and c == 1):
                                pass
                            else:
                                nc.scalar.dma_start(
                                    xT[c][:, hs], xT_d[c * P : (c + 1) * P, hs]
                                )

                # wv: halves on sync/scalar after xT; wp on scalar; mask last.
                half = (n_ct + 1) // 2
                nc.sync.dma_start(
                    wv[:, :half, :], wv_d[:half].rearrange("c p e -> p c e")
                )
                nc.scalar.dma_start(
                    wv[:, half:, :], wv_d[half:].rearrange("c p e -> p c e")
                )
                nc.scalar.dma_start(wp[:], wp_d.rearrange("c p e -> p c e"))
                nc.sync.dma_start(maskT[:], maskT_d[:])

                # qT/kT: psum[e-tile, t-chunk] = sum_c w_qk[c, e].T @ xT[c, t]
                # Opening: e-groups 0-3 in two c-OUTER passes over the xT
                # halves (pass A: tc 0-1 needs only each tile's first 1024
                # cols, pass B: tc 2-3).  4 e-groups x 2 chunks = 8 PSUM
                # banks per pass, and each arriving 256KB half feeds 4096
                # PE cycles (~150 GB/s demand, matching 2-queue delivery).
                for h in range(2):
                    pss = [
                        [
                            ph1ps.tile([P, TCH], FP32, name="qkps01", tag="qkps")
                            for _ in range(2)
                        ]
                        for _ in range(4)
                    ]
                    for c in range(n_ct):
                        for e in range(4):
                            for ti in range(2):
                                tc_i = 2 * h + ti
                                nc.tensor.matmul(
                                    pss[e][ti][:],
                                    wqk[:, c, e * P : (e + 1) * P],
                                    xT[c][:, tc_i * TCH : (tc_i + 1) * TCH],
                                    start=(c == 0),
                                    stop=(c == n_ct - 1),
                                )
                    for e in range(4):
                        for ti in range(2):
                            tc_i = 2 * h + ti
                            dst_ap = qT[e][:, tc_i * TCH : (tc_i + 1) * TCH]
                            if (e * 4 + tc_i) % 2 == 0:
                                nc.vector.tensor_copy(dst_ap, pss[e][ti][:])
                            else:
                                nc.scalar.copy(dst_ap, pss[e][ti][:])

                for e in range(4, 2 * N_ET):
                    dst = qT[e] if e < N_ET else kT[e - N_ET]
                    pss = [
                        ph1ps.tile([P, TCH], FP32, name="qkps", tag="qkps")
                        for _ in range(T // TCH)
                    ]
                    for c in range(n_ct):
                        for tc_i in range(T // TCH):
                            nc.tensor.matmul(
                                pss[tc_i][:],
                                wqk[:, c, e * P : (e + 1) * P],
                                xT[c][:, tc_i * TCH : (tc_i + 1) * TCH],
                                start=(c == 0),
                                stop=(c == n_ct - 1),
                            )
                    for tc_i in range(T // TCH):
                        dst_ap = dst[:, tc_i * TCH : (tc_i + 1) * TCH]
                        if (e * 4 + tc_i) % 2 == 0:
                            nc.vector.tensor_copy(dst_ap, pss[tc_i][:])
                        else:
                            nc.scalar.copy(dst_ap, pss[tc_i][:])

                # v: psum[t-tile, c'-chunk] = sum_c xT[c, t].T @ w_v[c, c']
                for t in range(N_TT):
                    pss = [
                        ph1ps.tile([P, TCH], FP32, name="vps", tag="qkps")
                        for _ in range(C // TCH)
                    ]
                    for c in range(n_ct):
                        for cc in range(C // TCH):
                            nc.tensor.matmul(
                                pss[cc][:],
                                xT[c][:, t * P : (t + 1) * P],
                                wv[:, c, cc * TCH : (cc + 1) * TCH],
                                start=(c == 0),
                                stop=(c == n_ct - 1),
                            )
                    for cc in range(C // TCH):
                        dst_ap = v[t][:, cc * TCH : (cc + 1) * TCH]
                        if (t * 2 + cc) % 2 == 0:
                            nc.vector.tensor_copy(dst_ap, pss[cc][:])
                        else:
                            nc.scalar.copy(dst_ap, pss[cc][:])

            # ---------------- phase 2: attention + proj ----------------
            with (
                tc.tile_pool(name="pt_pool", bufs=18) as pt_pool,
                tc.tile_pool(name="ot_pool", bufs=3) as ot_pool,
                tc.tile_pool(name="stage", bufs=3) as stage,
                tc.tile_pool(name="st_ps", bufs=2, space="PSUM") as st_ps,
                tc.tile_pool(name="sums_ps", bufs=1, space="PSUM") as sums_ps,
                tc.tile_pool(name="ot_ps", bufs=2, space="PSUM") as ot_ps,
                tc.tile_pool(name="pr_ps", bufs=3, space="PSUM") as pr_ps,
            ):
                def emit_proj(t0, ot_sb, rt, final=False):
                    """proj for the supertile starting at t0, scaled by 1/sums.
                    dch-outer so each 512-wide output chunk's copy+DMA starts
                    while the next chunk's matmuls still run.  The last chunk's
                    DMA rides the low-latency HWDGE queues (split across
                    sync+scalar on the final call) so the drain tail is short."""
                    for k in range(TCH // P):  # t-tile within supertile
                        for dch in range(C // TCH):
                            prs = pr_ps.tile([P, TCH], FP32, name="pr", tag="pr")
                            for g in range(N_ET):
                                nc.tensor.matmul(
                                    prs[:],
                                    ot_sb[g][:, k * P : (k + 1) * P],
                                    wp[:, g, dch * TCH : (dch + 1) * TCH],
                                    start=(g == 0),
                                    stop=(g == N_ET - 1),
                                )
                            osb_out = stage.tile([P, TCH], BF16, name="osb_out", tag="osb_out")
                            if dch % 2 == 0:
                                nc.scalar.activation(
                                    osb_out[:],
                                    prs[:],
                                    mybir.ActivationFunctionType.Copy,
                                    scale=rt[:, k : k + 1],
                                )
                            else:
                                nc.vector.tensor_scalar_mul(
                                    osb_out[:], prs[:], rt[:, k : k + 1]
                                )
                            r0, r1 = t0 + k * P, t0 + (k + 1) * P
                            c0 = dch * TCH
                            if final and k == TCH // P - 1 and dch == C // TCH - 1:
                                nc.sync.dma_start(
                                    out_d[r0:r1, c0 : c0 + 256], osb_out[:, :256]
                                )
                                nc.scalar.dma_start(
                                    out_d[r0:r1, c0 + 256 : c0 + TCH],
                                    osb_out[:, 256:],
                                )
                            else:
                                dq = nc.gpsimd if dch % 2 == 0 else nc.sync
                                dq.dma_start(
                                    out_d[r0:r1, c0 : c0 + TCH], osb_out[:]
                                )

                pending = None  # (t0, ot_sb, rt) of the previous supertile
                for i in range(N_SUP):  # supertile: t in [i*TCH, (i+1)*TCH)
                    t0 = i * TCH
                    n_st = 4 * i + 4  # causal s-tiles
                    ptiles = []
                    # fp32 running sum of the P~T tiles, built on the (idle)
                    # vector engine in the shadow of the ST matmuls; replaces
                    # the per-s-tile ones-matmuls (17.4k PE cycles total with
                    # a single 512-cycle matmul per supertile).
                    sacc = stage.tile([P, TCH], FP32, name="sacc", tag="sacc")
                    # --- ST + exp per s-tile ---
                    for j in range(n_st):
                        off = max(0, j - 4 * i) * P  # first valid t column
                        st = st_ps.tile([P, TCH], FP32, name="st", tag="st")
                        for e in range(N_ET):
                            nc.tensor.matmul(
                                st[:, off:TCH],
                                kT[e][:, j * P : (j + 1) * P],
                                qT[e][:, t0 + off : t0 + TCH],
                                start=(e == 0),
                                stop=(e == N_ET - 1),
                            )
                        if j >= 4 * i:  # diagonal block: strict-upper (s>t) mask
                            nc.vector.tensor_add(
                                st[:, off : off + P], st[:, off : off + P], maskT[:]
                            )
                        pt = pt_pool.tile([P, TCH], BF16, name="pt", tag="pt")
                        nc.scalar.activation(
                            pt[:, off:TCH],
                            st[:, off:TCH],
                            mybir.ActivationFunctionType.Exp,
                            scale=SCALE,
                        )
                        if j == 0:  # j=0 has off=0: full-width init
                            nc.vector.tensor_copy(sacc[:], pt[:])
                        else:
                            nc.vector.tensor_add(
                                sacc[:, off:TCH], sacc[:, off:TCH], pt[:, off:TCH]
                            )
                        ptiles.append((pt, off))

                    # --- previous supertile's proj (hides the recip roundtrip) ---
                    if pending is not None:
                        emit_proj(*pending)

                    # --- OT[c'-tile, t-chunk] = sum_s v[s,c'].T @ P~T[s,t] ---
                    # The row-sums block (one 128->1 ones-matmul over the bf16
                    # cast of sacc + DMA-roundtrip reciprocal) is emitted after
                    # OT's first group: the exps are all consumed by g=0
                    # anyway, so the PE never waits on the vector chain, and
                    # the roundtrip hides under the remaining 7 OT groups.
                    ot_sb = []
                    rt = None
                    for g in range(N_ET):
                        ot = ot_ps.tile([P, TCH], FP32, name="ot", tag="ot")
                        for j in range(n_st):
                            pt, off = ptiles[j]
                            nc.tensor.matmul(
                                ot[:, off:TCH],
                                v[j][:, g * P : (g + 1) * P],
                                pt[:, off:TCH],
                                start=(j == 0),
                                stop=(j == n_st - 1),
                            )
                        osb = ot_pool.tile([P, TCH], BF16, name="osb", tag=f"osb{g % 3}")
                        nc.vector.tensor_copy(osb[:], ot[:])
                        ot_sb.append(osb)
                        if g == 0:
                            sacc_bf = stage.tile([P, TCH], BF16, name="sacc_bf", tag="sacc_bf")
                            nc.vector.tensor_copy(sacc_bf[:], sacc[:])
                            sums = sums_ps.tile([1, TCH], FP32, name="sums", tag="sums")
                            nc.tensor.matmul(
                                sums[:], ones[:], sacc_bf[:], start=True, stop=True
                            )
                            srow = stage.tile([1, TCH], FP32, name="srow", tag="srow")
                            nc.vector.tensor_copy(srow[:], sums[:])
                            nc.sync.dma_start(scr_d[i, 0:1, :], srow[:])
                            rt0 = stage.tile([P, N_SUP], FP32, name="rt0", tag="rt0")
                            nc.sync.dma_start(
                                rt0[:], scr_d[i, 0].rearrange("(f q) -> q f", q=P)
                            )
                            rt = stage.tile([P, N_SUP], FP32, name="rt", tag="rt")
                            nc.vector.reciprocal(rt[:], rt0[:])

                    pending = (t0, ot_sb, rt)

                emit_proj(*pending, final=True)

    nc.compile()
    return nc


def kernel(x, w_attn, b_attn, w_proj, b_proj):
    x = np.asarray(x, dtype=np.float32)
    w_attn = np.asarray(w_attn, dtype=np.float32)
    b_attn = np.asarray(b_attn, dtype=np.float32)
    w_proj = np.asarray(w_proj, dtype=np.float32)
    b_proj = np.asarray(b_proj, dtype=np.float32)
    assert x.shape == (B, T, C)

    aug = bool(np.any(b_attn != 0.0))
    n_ct = C // P + (1 if aug else 0)
    if n_ct not in _cache:
        _cache[n_ct] = _build(n_ct)
    nc = _cache[n_ct]

    bf = ml_dtypes.bfloat16
    if aug:
        wqk = np.zeros((n_ct, P, 2 * C), dtype=bf)
        wqk.reshape(n_ct * P, 2 * C)[:C] = w_attn[:, : 2 * C].astype(bf)
        wqk.reshape(n_ct * P, 2 * C)[C] = b_attn[: 2 * C].astype(bf)
        wv = np.zeros((n_ct, P, C), dtype=bf)
        wv.reshape(n_ct * P, C)[:C] = w_attn[:, 2 * C :].astype(bf)
        wv.reshape(n_ct * P, C)[C] = b_attn[2 * C :].astype(bf)
    else:
        wqk = np.ascontiguousarray(w_attn[:, : 2 * C]).astype(bf).reshape(n_ct, P, 2 * C)
        wv = np.ascontiguousarray(w_attn[:, 2 * C :]).astype(bf).reshape(n_ct, P, C)
    wp = w_proj.astype(bf).reshape(N_ET, P, C)

    # strict upper triangle (s > t) additive mask for transposed [s, t] blocks
    maskT = np.where(
        np.arange(P)[:, None] > np.arange(P)[None, :], np.float32(NEG), np.float32(0.0)
    ).astype(np.float32)

    in_maps = []
    for b in range(B):
        xT = np.ascontiguousarray(x[b].T).astype(bf)
        if aug:
            xTa = np.zeros((n_ct * P, T), dtype=bf)
            xTa[:C] = xT
            xTa[C] = bf(1.0)
            xT = xTa
        in_maps.append({"xT": xT, "wqk": wqk, "wv": wv, "wp": wp, "maskT": maskT})

    global _last_in_maps
    _last_in_maps = in_maps
    res = run_bass_kernel_spmd(nc, in_maps, core_ids=list(range(8)))
    out = np.stack([res.results[b]["out"] for b in range(B)]).astype(np.float32)
    if np.any(b_proj != 0.0):
        out = out + b_proj[None, None, :]
    return out


if __name__ == "__main__":
    rng = np.random.default_rng(0)
    x = rng.standard_normal((B, T, C), dtype=np.float32)
    w_attn = rng.standard_normal((C, 3 * C), dtype=np.float32) / np.sqrt(C)
    b_attn = np.zeros(3 * C, dtype=np.float32)
    w_proj = rng.standard_normal((C, C), dtype=np.float32) / np.sqrt(C)
    b_proj = np.zeros(C, dtype=np.float32)
    out = kernel(x, w_attn, b_attn, w_proj, b_proj)
    print(out.shape, out.dtype)


# revision 18
# speedup vs baseline: 1.0520x; 1.0008x over previous
"""Causal self-attention kernel for Trainium2 (8 NeuronCores, data-parallel).

Problem: B=8, T=2048, C=1024 single-head causal attention:
    qkv = x @ w_attn + b_attn ; q,k,v = split(qkv)
    attn = softmax(q @ k.T / sqrt(C) + causal_mask)
    out  = (attn @ v) @ w_proj + b_proj

Sharding: pure data parallel — one batch element per core, weights replicated,
no collectives.

Per-core algorithm (all matmuls bf16 operands, fp32 PSUM accumulate):
  host: xT = x[b].T cast bf16 (so the contraction dim is the partition dim
        everywhere on device; no on-device transposes needed anywhere).
  ph1:  qT[e,t], kT[e,s]  <- matmul(lhsT=w_qk[c,e-tile], rhs=xT[c,t])   [e,t] layout
        v[t,c']           <- matmul(lhsT=xT[c,t-tile],  rhs=w_v[c,c'])  natural layout
  ph2:  per 512-wide t-chunk ("supertile"), per 128-wide s-tile (causal only):
        ST[s,t]  <- matmul(lhsT=kT[e,s-tile], rhs=qT[e,t-chunk])  (8 e-tiles acc)
        P~T[s,t] <- exp(ST/sqrt(C) + mask)   (no max-subtract; logits are O(1))
        sums[t]  <- matmul(lhsT=ones[s,1], rhs=P~T)  (acc over s-tiles)
        OT[c',t] <- matmul(lhsT=v[s-tile,c'-tile], rhs=P~T[s-tile,t-chunk])
        out[t,d] <- matmul(lhsT=OT[c',t-tile], rhs=w_proj[c',d]) * (1/sums[t])
  The 1/sums normalization is folded into the final PSUM->SBUF copy as a
  per-partition activation scale (everything between exp and out is linear).
  The proj stage runs one supertile behind (software pipeline) so the
  sums->reciprocal DMA roundtrip never stalls the PE.

DMA plan (3 queues: sync + scalar are HWDGE, gpsimd is SWDGE):
  - xT c-tiles split by parity across sync/scalar, each tile DMA'd as two
    1024-col halves so the first matmuls can start ~4us in.
  - weights live in single 3-dim SBUF tiles ([P, c, cols]) so each eb-block
    is ONE big descriptor (per-descriptor issue is ~0.65us on a queue;
    32 small descriptors would serialize for 20us+).
  - wqk eb0 goes per-c-tile first (matches the c-outer e-group 0-1 ramp
    ordering), the rest as 1MB descriptors.
  - No PE warmup: the Tensor queue's framework preamble ends ~6.5us anyway,
    by which time the first xT/wqk blocks have landed; the p-state clock
    ramps during the DMA-limited opening instead of on dummy matmuls.

Output is written bf16 (halves the out DMA) and upcast on host.

b_attn is folded in by augmenting x with a ones column (padded to a full
128-partition tile) only when it is nonzero; b_proj is added on the host.
"""

import sys

if "/opt/trn_rl_repo" not in sys.path:
    sys.path.insert(0, "/opt/trn_rl_repo")

import numpy as np
import ml_dtypes

import concourse.bacc as bacc
import concourse.mybir as mybir
import concourse.tile as tile
from concourse.bass_utils import run_bass_kernel_spmd

B, T, C = 8, 2048, 1024
P = 128  # partitions
TCH = 512  # t-chunk (moving free dim)
N_TT = T // P  # 16 t-tiles
N_SUP = T // TCH  # 4 supertiles
N_ET = C // P  # 8 e-tiles (q/k feature dim)
SCALE = 1.0 / float(np.sqrt(np.float32(C)))
NEG = -10000000000.0

BF16 = mybir.dt.bfloat16
FP32 = mybir.dt.float32

_cache = {}


def _build(n_ct):
    """Build the SPMD Bass program. n_ct = number of 128-wide c-tiles of the
    (possibly ones-augmented) input feature dim."""
    nc = bacc.Bacc("TRN2", target_bir_lowering=False, debug=False, num_devices=8)

    xT_d = nc.dram_tensor("xT", [n_ct * P, T], BF16, kind="ExternalInput").ap()
    wqk_d = nc.dram_tensor("wqk", [n_ct, P, 2 * C], BF16, kind="ExternalInput").ap()
    wv_d = nc.dram_tensor("wv", [n_ct, P, C], BF16, kind="ExternalInput").ap()
    wp_d = nc.dram_tensor("wp", [N_ET, P, C], BF16, kind="ExternalInput").ap()
    maskT_d = nc.dram_tensor("maskT", [P, P], FP32, kind="ExternalInput").ap()
    out_d = nc.dram_tensor("out", [T, C], BF16, kind="ExternalOutput").ap()
    scr_d = nc.dram_tensor("scr", [N_SUP, 4, TCH], FP32, kind="ExternalOutput").ap()

    with tile.TileContext(nc) as tc:
        with (
            tc.tile_pool(name="persist", bufs=1) as persist,
            tc.tile_pool(name="small", bufs=1) as small,
        ):
            ones = small.tile([P, 1], BF16, name="ones", tag="ones")
            nc.vector.memset(ones[:], 1.0)

            # persistent SBUF arrays
            qT = [persist.tile([P, T], BF16, name=f"qT{e}", tag=f"qT{e}") for e in range(N_ET)]
            kT = [persist.tile([P, T], BF16, name=f"kT{e}", tag=f"kT{e}") for e in range(N_ET)]
            v = [persist.tile([P, C], BF16, name=f"v{t}", tag=f"v{t}") for t in range(N_TT)]
            maskT = small.tile([P, P], FP32, name="maskT", tag="maskT")
            # w_proj persists into phase 2; loaded early on the scalar queue.
            wp = persist.tile([P, N_ET, C], BF16, name="wp", tag="wp")

            # PE warmup: ~26 matmuls over (uninitialized) maskT garbage keep
            # the PE busy from the end of the framework preamble (~7us) until
            # the first input descriptors complete (~10us), ramping the HAM
            # clock to 2.4 GHz.  Reading a not-yet-written tile means the
            # first matmul has NO semaphore wait (the maskT DMA is ordered
            # after these reads, and it isn't needed until phase 2 anyway);
            # the outputs land in a scratch PSUM bank and are discarded.
            mbf = maskT[:].bitcast(BF16)
            with tc.tile_pool(name="warm_ps", bufs=1, space="PSUM") as warm_ps:
                wps = warm_ps.tile([1, 256], FP32, name="wps", tag="wps")
                NWARM = 26
                for i in range(NWARM):
                    nc.tensor.matmul(
                        wps[:], mbf[:, 0:1], mbf[:, 0:256],
                        start=(i == 0), stop=(i == NWARM - 1),
                    )

            # ---------------- phase 1: projections ----------------
            with (
                tc.tile_pool(name="ph1", bufs=1) as ph1,
                tc.tile_pool(name="ph1ps", bufs=8, space="PSUM") as ph1ps,
            ):
                xT = [ph1.tile([P, T], BF16, name=f"xT{c}", tag=f"xT{c}") for c in range(n_ct)]
                wqk = ph1.tile([P, n_ct, 2 * C], BF16, name="wqk", tag="wqk")
                wv = ph1.tile([P, n_ct, C], BF16, name="wv", tag="wv")

                # gpsimd (SWDGE): wqk eb0 (cols 0:512 — everything the opening
                # passes' e-groups 0-3 touch) per-c first, then big blocks.
                # Two late xT h1 halves ride on gpsimd between eb0 and eb1 to
                # balance the three ~90-100 GB/s queues (input load is
                # HBM-BW-bound at ~350 GB/s aggregate).
                # ladder: c0's block split so the very first matmuls unblock
                # one descriptor-latency earlier
                nc.gpsimd.dma_start(wqk[:, 0, :256], wqk_d[0, :, :256])
                nc.gpsimd.dma_start(wqk[:, 0, 256:TCH], wqk_d[0, :, 256:TCH])
                for c in range(1, n_ct):
                    nc.gpsimd.dma_start(wqk[:, c, :TCH], wqk_d[c, :, :TCH])
                h1 = slice(T // 2, T)
                gp_x = [c for c in range(n_ct) if c >= 6]
                for c in gp_x:
                    nc.gpsimd.dma_start(xT[c][:, h1], xT_d[c * P : (c + 1) * P, h1])
                for eb in range(1, 2 * C // TCH):
                    nc.gpsimd.dma_start(
                        wqk[:, :, eb * TCH : (eb + 1) * TCH],
                        wqk_d[:, :, eb * TCH : (eb + 1) * TCH].rearrange(
                            "c p e -> p c e"
                        ),
                    )

                # xT: parity split across sync/scalar HWDGE queues, two
                # 1024-col halves per c-tile; all h0 halves (pass A) first.
                ch = [c for c in range(n_ct) if c % 2 == 0]
                co = [c for c in range(n_ct) if c % 2 == 1]
                # ladder: first halves of c0/c1 as two 512-col descriptors
                nc.sync.dma_start(xT[0][:, :TCH], xT_d[0:P, :TCH])
                nc.sync.dma_start(xT[0][:, TCH : T // 2], xT_d[0:P, TCH : T // 2])
                if n_ct > 1:
                    nc.scalar.dma_start(xT[1][:, :TCH], xT_d[P : 2 * P, :TCH])
                    nc.scalar.dma_start(
                        xT[1][:, TCH : T // 2], xT_d[P : 2 * P, TCH : T // 2]
                    )
                for h in range(2):
                    hs = slice(h * (T // 2), (h + 1) * (T // 2))
                    for i in range(max(len(ch), len(co))):
                        if i < len(ch):
                            c = ch[i]
                            if (h == 1 and c in gp_x) or (h == 0 and c == 0):
                                pass
                            else:
                                nc.sync.dma_start(
                                    xT[c][:, hs], xT_d[c * P : (c + 1) * P, hs]
                                )
                        if i < len(co):
                            c = co[i]
                            if (h == 1 and c in gp_x) or (h == 0 and c == 1):
                                pass
                            else:
                                nc.scalar.dma_start(
                                    xT[c][:, hs], xT_d[c * P : (c + 1) * P, hs]
                                )

                # wv: halves on sync/scalar after xT; wp on scalar; mask last.
                half = (n_ct + 1) // 2
                nc.sync.dma_start(
                    wv[:, :half, :], wv_d[:half].rearrange("c p e -> p c e")
                )
                nc.scalar.dma_start(
                    wv[:, half:, :], wv_d[half:].rearrange("c p e -> p c e")
                )
                nc.scalar.dma_start(wp[:], wp_d.rearrange("c p e -> p c e"))
                nc.sync.dma_start(maskT[:], maskT_d[:])

                # qT/kT: psum[e-tile, t-chunk] = sum_c w_qk[c, e].T @ xT[c, t]
                # Opening: e-groups 0-3 in two c-OUTER passes over the xT
                # halves (pass A: tc 0-1 needs only each tile's first 1024
                # cols, pass B: tc 2-3).  4 e-groups x 2 chunks = 8 PSUM
                # banks per pass, and each arriving 256KB half feeds 4096
                # PE cycles (~150 GB/s demand, matching 2-queue delivery).
                for h in range(2):
                    pss = [
                        [
                            ph1ps.tile([P, TCH], FP32, name="qkps01", tag="qkps")
                            for _ in range(2)
                        ]
                        for _ in range(4)
                    ]
                    for c in range(n_ct):
                        for e in range(4):
                            for ti in range(2):
                                tc_i = 2 * h + ti
                                nc.tensor.matmul(
                                    pss[e][ti][:],
                                    wqk[:, c, e * P : (e + 1) * P],
                                    xT[c][:, tc_i * TCH : (tc_i + 1) * TCH],
                                    start=(c == 0),
                                    stop=(c == n_ct - 1),
                                )
                    for e in range(4):
                        for ti in range(2):
                            tc_i = 2 * h + ti
                            dst_ap = qT[e][:, tc_i * TCH : (tc_i + 1) * TCH]
                            if (e * 4 + tc_i) % 2 == 0:
                                nc.vector.tensor_copy(dst_ap, pss[e][ti][:])
                            else:
                                nc.scalar.copy(dst_ap, pss[e][ti][:])

                for e in range(4, 2 * N_ET):
                    dst = qT[e] if e < N_ET else kT[e - N_ET]
                    pss = [
                        ph1ps.tile([P, TCH], FP32, name="qkps", tag="qkps")
                        for _ in range(T // TCH)
                    ]
                    for c in range(n_ct):
                        for tc_i in range(T // TCH):
                            nc.tensor.matmul(
                                pss[tc_i][:],
                                wqk[:, c, e * P : (e + 1) * P],
                                xT[c][:, tc_i * TCH : (tc_i + 1) * TCH],
                                start=(c == 0),
                                stop=(c == n_ct - 1),
                            )
                    for tc_i in range(T // TCH):
                        dst_ap = dst[:, tc_i * TCH : (tc_i + 1) * TCH]
                        if (e * 4 + tc_i) % 2 == 0:
                            nc.vector.tensor_copy(dst_ap, pss[tc_i][:])
                        else:
                            nc.scalar.copy(dst_ap, pss[tc_i][:])

                # v: psum[t-tile, c'-chunk] = sum_c xT[c, t].T @ w_v[c, c']
                for t in range(N_TT):
                    pss = [
                        ph1ps.tile([P, TCH], FP32, name="vps", tag="qkps")
                        for _ in range(C // TCH)
                    ]
                    for c in range(n_ct):
                        for cc in range(C // TCH):
                            nc.tensor.matmul(
                                pss[cc][:],
                                xT[c][:, t * P : (t + 1) * P],
                                wv[:, c, cc * TCH : (cc + 1) * TCH],
                                start=(c == 0),
                                stop=(c == n_ct - 1),
                            )
                    for cc in range(C // TCH):
                        dst_ap = v[t][:, cc * TCH : (cc + 1) * TCH]
                        if (t * 2 + cc) % 2 == 0:
                            nc.vector.tensor_copy(dst_ap, pss[cc][:])
                        else:
                            nc.scalar.copy(dst_ap, pss[cc][:])

            # ---------------- phase 2: attention + proj ----------------
            with (
                tc.tile_pool(name="pt_pool", bufs=18) as pt_pool,
                tc.tile_pool(name="ot_pool", bufs=3) as ot_pool,
                tc.tile_pool(name="stage", bufs=3) as stage,
                tc.tile_pool(name="st_ps", bufs=2, space="PSUM") as st_ps,
                tc.tile_pool(name="sums_ps", bufs=1, space="PSUM") as sums_ps,
                tc.tile_pool(name="ot_ps", bufs=2, space="PSUM") as ot_ps,
                tc.tile_pool(name="pr_ps", bufs=3, space="PSUM") as pr_ps,
            ):
                def emit_proj(t0, ot_sb, rt, final=False):
                    """proj for the supertile starting at t0, scaled by 1/sums.
                    dch-outer so each 512-wide output chunk's copy+DMA starts
                    while the next chunk's matmuls still run.  The last chunk's
                    DMA rides the low-latency HWDGE queues (split across
                    sync+scalar on the final call) so the drain tail is short."""
                    for k in range(TCH // P):  # t-tile within supertile
                        for dch in range(C // TCH):
                            prs = pr_ps.tile([P, TCH], FP32, name="pr", tag="pr")
                            for g in range(N_ET):
                                nc.tensor.matmul(
                                    prs[:],
                                    ot_sb[g][:, k * P : (k + 1) * P],
                                    wp[:, g, dch * TCH : (dch + 1) * TCH],
                                    start=(g == 0),
                                    stop=(g == N_ET - 1),
                                )
                            osb_out = stage.tile([P, TCH], BF16, name="osb_out", tag="osb_out")
                            if dch % 2 == 0:
                                nc.scalar.activation(
                                    osb_out[:],
                                    prs[:],
                                    mybir.ActivationFunctionType.Copy,
                                    scale=rt[:, k : k + 1],
                                )
                            else:
                                nc.vector.tensor_scalar_mul(
                                    osb_out[:], prs[:], rt[:, k : k + 1]
                                )
                            r0, r1 = t0 + k * P, t0 + (k + 1) * P
                            c0 = dch * TCH
                            if final and k == TCH // P - 1 and dch == C // TCH - 1:
                                nc.sync.dma_start(
                                    out_d[r0:r1, c0 : c0 + 256], osb_out[:, :256]
                                )
                                nc.scalar.dma_start(
                                    out_d[r0:r1, c0 + 256 : c0 + TCH],
                                    osb_out[:, 256:],
                                )
                            else:
                                dq = nc.gpsimd if dch % 2 == 0 else nc.sync
                                dq.dma_start(
                                    out_d[r0:r1, c0 : c0 + TCH], osb_out[:]
                                )

                pending = None  # (t0, ot_sb, rt) of the previous supertile
                for i in range(N_SUP):  # supertile: t in [i*TCH, (i+1)*TCH)
                    t0 = i * TCH
                    n_st = 4 * i + 4  # causal s-tiles
                    ptiles = []
                    # fp32 running sum of the P~T tiles, built on the (idle)
                    # vector engine in the shadow of the ST matmuls; replaces
                    # the per-s-tile ones-matmuls (17.4k PE cycles total with
                    # a single 512-cycle matmul per supertile).
                    sacc = stage.tile([P, TCH], FP32, name="sacc", tag="sacc")
                    # --- ST + exp per s-tile ---
                    for j in range(n_st):
                        off = max(0, j - 4 * i) * P  # first valid t column
                        st = st_ps.tile([P, TCH], FP32, name="st", tag="st")
                        for e in range(N_ET):
                            nc.tensor.matmul(
                                st[:, off:TCH],
                                kT[e][:, j * P : (j + 1) * P],
                                qT[e][:, t0 + off : t0 + TCH],
                                start=(e == 0),
                                stop=(e == N_ET - 1),
                            )
                        if j >= 4 * i:  # diagonal block: strict-upper (s>t) mask
                            nc.vector.tensor_add(
                                st[:, off : off + P], st[:, off : off + P], maskT[:]
                            )
                        pt = pt_pool.tile([P, TCH], BF16, name="pt", tag="pt")
                        nc.scalar.activation(
                            pt[:, off:TCH],
                            st[:, off:TCH],
                            mybir.ActivationFunctionType.Exp,
                            scale=SCALE,
                        )
                        if j == 0:  # j=0 has off=0: full-width init
                            nc.vector.tensor_copy(sacc[:], pt[:])
                        else:
                            nc.vector.tensor_add(
                                sacc[:, off:TCH], sacc[:, off:TCH], pt[:, off:TCH]
                            )
                        ptiles.append((pt, off))

                    # --- previous supertile's proj (hides the recip roundtrip) ---
                    if pending is not None:
                        emit_proj(*pending)

                    # --- OT[c'-tile, t-chunk] = sum_s v[s,c'].T @ P~T[s,t] ---
                    # The row-sums block (one 128->1 ones-matmul over the bf16
                    # cast of sacc + DMA-roundtrip reciprocal) is emitted after
                    # OT's first group: the exps are all consumed by g=0
                    # anyway, so the PE never waits on the vector chain, and
                    # the roundtrip hides under the remaining 7 OT groups.
                    ot_sb = []
                    rt = None
                    for g in range(N_ET):
                        ot = ot_ps.tile([P, TCH], FP32, name="ot", tag="ot")
                        for j in range(n_st):
                            pt, off = ptiles[j]
                            nc.tensor.matmul(
                                ot[:, off:TCH],
                                v[j][:, g * P : (g + 1) * P],
                                pt[:, off:TCH],
                                start=(j == 0),
                                stop=(j == n_st - 1),
                            )
                        osb = ot_pool.tile([P, TCH], BF16, name="osb", tag=f"osb{g % 3}")
                        nc.vector.tensor_copy(osb[:], ot[:])
                        ot_sb.append(osb)
                        if g == 0:
                            sacc_bf = stage.tile([P, TCH], BF16, name="sacc_bf", tag="sacc_bf")
                            nc.vector.tensor_copy(sacc_bf[:], sacc[:])
                            sums = sums_ps.tile([1, TCH], FP32, name="sums", tag="sums")
                            nc.tensor.matmul(
                                sums[:], ones[:], sacc_bf[:], start=True, stop=True
                            )
                            srow = stage.tile([1, TCH], FP32, name="srow", tag="srow")
                            nc.vector.tensor_copy(srow[:], sums[:])
                            nc.sync.dma_start(scr_d[i, 0:1, :], srow[:])
                            rt0 = stage.tile([P, N_SUP], FP32, name="rt0", tag="rt0")
                            nc.sync.dma_start(
                                rt0[:], scr_d[i, 0].rearrange("(f q) -> q f", q=P)
                            )
                            rt = stage.tile([P, N_SUP], FP32, name="rt", tag="rt")
                            nc.vector.reciprocal(rt[:], rt0[:])

                    pending = (t0, ot_sb, rt)

                emit_proj(*pending, final=True)

    nc.compile()
    return nc


def kernel(x, w_attn, b_attn, w_proj, b_proj):
    x = np.asarray(x, dtype=np.float32)
    w_attn = np.asarray(w_attn, dtype=np.float32)
    b_attn = np.asarray(b_attn, dtype=np.float32)
    w_proj = np.asarray(w_proj, dtype=np.float32)
    b_proj = np.asarray(b_proj, dtype=np.float32)
    assert x.shape == (B, T, C)

    aug = bool(np.any(b_attn != 0.0))
    n_ct = C // P + (1 if aug else 0)
    if n_ct not in _cache:
        _cache[n_ct] = _build(n_ct)
    nc = _cache[n_ct]

    bf = ml_dtypes.bfloat16
    if aug:
        wqk = np.zeros((n_ct, P, 2 * C), dtype=bf)
        wqk.reshape(n_ct * P, 2 * C)[:C] = w_attn[:, : 2 * C].astype(bf)
        wqk.reshape(n_ct * P, 2 * C)[C] = b_attn[: 2 * C].astype(bf)
        wv = np.zeros((n_ct, P, C), dtype=bf)
        wv.reshape(n_ct * P, C)[:C] = w_attn[:, 2 * C :].astype(bf)
        wv.reshape(n_ct * P, C)[C] = b_attn[2 * C :].astype(bf)
    else:
        wqk = np.ascontiguousarray(w_attn[:, : 2 * C]).astype(bf).reshape(n_ct, P, 2 * C)
        wv = np.ascontiguousarray(w_attn[:, 2 * C :]).astype(bf).reshape(n_ct, P, C)
    wp = w_proj.astype(bf).reshape(N_ET, P, C)

    # strict upper triangle (s > t) additive mask for transposed [s, t] blocks
    maskT = np.where(
        np.arange(P)[:, None] > np.arange(P)[None, :], np.float32(NEG), np.float32(0.0)
    ).astype(np.float32)

    in_maps = []
    for b in range(B):
        xT = np.ascontiguousarray(x[b].T).astype(bf)
        if aug:
            xTa = np.zeros((n_ct * P, T), dtype=bf)
            xTa[:C] = xT
            xTa[C] = bf(1.0)
            xT = xTa
        in_maps.append({"xT": xT, "wqk": wqk, "wv": wv, "wp": wp, "maskT": maskT})

    global _last_in_maps
    _last_in_maps = in_maps
    res = run_bass_kernel_spmd(nc, in_maps, core_ids=list(range(8)))
    out = np.stack([res.results[b]["out"] for b in range(B)]).astype(np.float32)
    if np.any(b_proj != 0.0):
        out = out + b_proj[None, None, :]
    return out


if __name__ == "__main__":
    rng = np.random.default_rng(0)
    x = rng.standard_normal((B, T, C), dtype=np.float32)
    w_attn = rng.standard_normal((C, 3 * C), dtype=np.float32) / np.sqrt(C)
    b_attn = np.zeros(3 * C, dtype=np.float32)
    w_proj = rng.standard_normal((C, C), dtype=np.float32) / np.sqrt(C)
    b_proj = np.zeros(C, dtype=np.float32)
    out = kernel(x, w_attn, b_attn, w_proj, b_proj)
    print(out.shape, out.dtype)
